# revision 9
# baseline (speedup 1.0000x reference)
"""GAT encoder (4-head 256 concat + mu/logvar 128) on 8 trn2 cores — v2.

Strategy (dst-range node sharding):
 - Host sorts edges by dst, buckets per core (2500 dst nodes each), pads each
   128-node block to TPB tiles of 128 edges; 2048-edge gather chunks.
 - Phase A (per core, redundant): one fused bf16 matmul xTb @ [W1|Wss1|Wsd1]
   per 128-node tile -> XPT rows [xp bf16 0:256 | ss1 f32 | sd1 f32] (768B).
 - L1 edge phase: 768B dma_gather of XPT rows by src (payload+ss), 256B
   sub-row gather of XPT[256:384] by dst (sd); softmax coefs scaled into the
   payload in place; per 128-edge tile a one-hot bf16 matmul accumulates
   messages + ex hi/lo denominator columns into a per-block PSUM.
 - L1 finalize per block: normalize, +bias, ELU (bf16) -> h; PE-transpose and
   project with [Wmu|vmu|umu]/[Wlv|vlv|ulv]; rows [xpmu|xplv|logits f32] go to
   agin. AllGather is split into 4 chunks of 5 blocks, each issued as soon as
   its blocks are final so the collective overlaps the L1 tail.
 - L2/3 edge phase: 768B gather of agout rows by src, 256B sub-row gather of
   agin[256:384] by dst (prefetched into a compact sd table during the AG
   tail); same one-hot trick, mu and lv share one gather/matmul per tile.
Outputs (mu, logvar) assembled host-side from per-core slices.
"""

import numpy as np

# ---- problem constants (hardcoded per contract) ----
N = 20000
E = 320000
FIN = 512
HID = 256
LAT = 128
H = 4
C1 = 64
NEG = 0.2
EPS = 1e-16

NC = 8
NOWN = 2500          # dst nodes per core
BLOCKS = 20          # 128-node blocks per core
NLOC = BLOCKS * 128  # 2560
TPB = 18             # tiles (128 edges) per block (max real block = 2174 edges)
TILES = BLOCKS * TPB       # 360 real tiles
TPC = 8                   # tiles per gather chunk
CHUNK = TPC * 128
IC = CHUNK // 16           # idx table cols per chunk
NCHUNK = 45
TILES_PAD = NCHUNK * TPC   # 360
EPAD = TILES_PAD * 128     # 47104 edge slots per core
NPADA = 160 * 128          # 20480 padded global rows
XW = 384                   # XPT/agin/agout row width in bf16 slots (768B)
GAG = 4                    # AllGather split count (5 blocks each)
AG_EMIT = (6, 12, 17, 22)  # chunk after which AG_g is emitted (deps: 5,11,16,22)

_cache = {}


def _wrap_idxs(idx):
    n = idx.shape[0]
    t = np.zeros((128, n // 16), np.int16)
    w = idx.reshape(n // 16, 16).T.astype(np.int16)
    for g in range(8):
        t[g * 16:(g + 1) * 16, :] = w
    return t


def _colmajor(a):
    # per-edge array [EPAD] -> [128, TILES_PAD] tile-column layout
    return np.ascontiguousarray(a.reshape(TILES_PAD, 128).T)


def _build_module(upto="full"):
    import concourse.bacc as bacc
    import concourse.mybir as mybir
    import concourse.tile as tile

    f32 = mybir.dt.float32
    bf16 = mybir.dt.bfloat16
    i16 = mybir.dt.int16
    Alu = mybir.AluOpType
    Act = mybir.ActivationFunctionType

    nc = bacc.Bacc("TRN2", target_bir_lowering=False, num_devices=NC)

    # ---- inputs ----
    xTb = nc.dram_tensor("xTb", [FIN, NPADA], bf16, kind="ExternalInput")
    w1e = nc.dram_tensor("w1e", [FIN, 264], bf16, kind="ExternalInput")
    wmue = nc.dram_tensor("wmue", [HID, 130], bf16, kind="ExternalInput")
    wlve = nc.dram_tensor("wlve", [HID, 130], bf16, kind="ExternalInput")
    b1b = nc.dram_tensor("b1b", [128, 256], f32, kind="ExternalInput")
    bmub = nc.dram_tensor("bmub", [128, 128], f32, kind="ExternalInput")
    blvb = nc.dram_tensor("blvb", [128, 128], f32, kind="ExternalInput")
    iota = nc.dram_tensor("iota", [128, 128], bf16, kind="ExternalInput")
    ident = nc.dram_tensor("ident", [128, 128], f32, kind="ExternalInput")
    identb = nc.dram_tensor("identb", [128, 128], bf16, kind="ExternalInput")
    srcg = nc.dram_tensor("srcg", [128, EPAD // 16], i16, kind="ExternalInput")
    src2 = nc.dram_tensor("src2", [128, EPAD // 16], i16, kind="ExternalInput")
    dstl = nc.dram_tensor("dstl", [128, EPAD // 16], i16, kind="ExternalInput")
    dstoffT = nc.dram_tensor("dstoffT", [128, TILES_PAD], f32, kind="ExternalInput")
    wT = nc.dram_tensor("wT", [128, TILES_PAD], f32, kind="ExternalInput")

    mu_out = nc.dram_tensor("mu_out", [NLOC, LAT], f32, kind="ExternalOutput")
    lv_out = nc.dram_tensor("lv_out", [NLOC, LAT], f32, kind="ExternalOutput")

    with tile.TileContext(nc) as tc:
        with (
            tc.tile_pool(name="cst", bufs=1) as cst,
            tc.tile_pool(name="lw", bufs=3) as lw,
            tc.tile_pool(name="xa", bufs=3) as xa,
            tc.tile_pool(name="gx", bufs=3) as gx,
            tc.tile_pool(name="ge", bufs=3) as ge,
            tc.tile_pool(name="oh", bufs=20) as ohp,
            tc.tile_pool(name="sm", bufs=6) as sm,
            tc.tile_pool(name="fin", bufs=3) as fin,
            tc.tile_pool(name="ps2", bufs=3, space="PSUM") as ps2,
            tc.tile_pool(name="psa", bufs=2, space="PSUM") as psa,
            tc.tile_pool(name="ps1", bufs=1, space="PSUM") as ps1,
            tc.tile_pool(name="dr", bufs=1, space="DRAM") as dr,
        ):
            # internal DRAM tables (aginc is the contiguous collective
            # input; wide tables keep 768B rows for gathers)
            XPT = dr.tile([NPADA, XW], bf16, tag="XPT")
            SDT = dr.tile([NLOC, 128], bf16, tag="SDT")
            SD2T = dr.tile([NLOC, 128], bf16, tag="SD2T")
            agin = dr.tile([NLOC, XW], bf16, tag="agin")
            agincs = []
            for g in range(GAG):
                aginc_g = dr.tile([NLOC // GAG, 264], bf16,
                                  tag=f"aginc{g}", name=f"aginc{g}")
                agincs.append(aginc_g)
            agall = dr.tile([NC * NLOC, XW], bf16, tag="agall")
            agouts = []
            for g in range(GAG):
                agout_g = dr.tile([NC * (NLOC // GAG), 264], bf16,
                                  tag=f"agout{g}", name=f"agout{g}",
                                  addr_space="Shared")
                agouts.append(agout_g)

            # resident constants
            w1e_t = []
            for kk in range(4):
                t = cst.tile([128, 264], bf16, tag=f"w1e{kk}")
                nc.sync.dma_start(t[:], w1e[kk * 128:(kk + 1) * 128, :])
                w1e_t.append(t)
            wmue_t = []
            wlve_t = []
            for kk in range(2):
                t = cst.tile([128, 130], bf16, tag=f"wmue{kk}")
                nc.sync.dma_start(t[:], wmue[kk * 128:(kk + 1) * 128, :])
                wmue_t.append(t)
                t2 = cst.tile([128, 130], bf16, tag=f"wlve{kk}")
                nc.sync.dma_start(t2[:], wlve[kk * 128:(kk + 1) * 128, :])
                wlve_t.append(t2)
            b1b_t = cst.tile([128, 256], f32, tag="b1b")
            nc.sync.dma_start(b1b_t[:], b1b[:])
            bmub_t = cst.tile([128, 128], f32, tag="bmub")
            nc.sync.dma_start(bmub_t[:], bmub[:])
            blvb_t = cst.tile([128, 128], f32, tag="blvb")
            nc.sync.dma_start(blvb_t[:], blvb[:])
            iota_t = cst.tile([128, 128], bf16, tag="iota")
            nc.sync.dma_start(iota_t[:], iota[:])
            ident_t = cst.tile([128, 128], f32, tag="ident")
            nc.sync.dma_start(ident_t[:], ident[:])
            identb_t = cst.tile([128, 128], bf16, tag="identb")
            nc.sync.dma_start(identb_t[:], identb[:])
            srcg_t = cst.tile([128, EPAD // 16], i16, tag="srcg")
            nc.sync.dma_start(srcg_t[:], srcg[:])
            src2_t = cst.tile([128, EPAD // 16], i16, tag="src2")
            nc.sync.dma_start(src2_t[:], src2[:])
            dstl_t = cst.tile([128, EPAD // 16], i16, tag="dstl")
            nc.sync.dma_start(dstl_t[:], dstl[:])
            dstoffT_t = cst.tile([128, TILES_PAD], f32, tag="dstoffT")
            nc.sync.dma_start(dstoffT_t[:], dstoffT[:])
            wT_t = cst.tile([128, TILES_PAD], f32, tag="wT")
            nc.sync.dma_start(wT_t[:], wT[:])
            # compact per-edge dst logits for L2/3, filled during the AG tail
            sdall = cst.tile([128, TILES_PAD, 8], bf16, tag="sdall")
            # compact per-edge dst logits for L1, filled during phase A
            sdall1 = cst.tile([128, TILES_PAD, 8], bf16, tag="sdall1")

            # ---- phase A (replicated, own-first row permutation): one
            # fused matmul per 128-node tile -> XPT rows; own rows land in
            # groups 0-4 so L1 dst-logit gathers overlap later groups ----
            pre1_done = 0
            for g in range(NPADA // 512):
                lx = lw.tile([128, 4, 512], bf16, tag="lx")
                nc.sync.dma_start(
                    lx[:], xTb[:].rearrange("(kk p) (g n) -> p kk g n",
                                            p=128, n=512)[:, :, g, :])
                xps = xa.tile([128, 4, 272], bf16, tag="xps")
                for ti in range(4):
                    ps = psa.tile([128, 264], f32, tag="psA", name="psA")
                    for kk in range(4):
                        sl = slice(ti * 128, (ti + 1) * 128)
                        nc.tensor.matmul(ps[:], lx[:, kk, sl], w1e_t[kk][:],
                                         start=(kk == 0), stop=(kk == 3))
                    nc.scalar.copy(xps[:, ti, 0:256], ps[:, 0:256])
                    nc.vector.tensor_copy(
                        xps[:, ti, 256:272].bitcast(f32), ps[:, 256:264])
                nc.sync.dma_start(
                    XPT[:].rearrange("(g4 p) c -> p g4 c", p=128)
                    [:, 4 * g:4 * g + 4, 0:272], xps[:])
                if g < 5:
                    nc.sync.dma_start(
                        SDT[:].rearrange("(g4 p) c -> p g4 c", p=128)
                        [:, 4 * g:4 * g + 4, 0:16], xps[:, :, 256:272])
                if g >= 6 and upto != "A":
                    for _ in range(2):
                        if pre1_done >= NCHUNK:
                            break
                        ci1 = pre1_done
                        ext1 = ge.tile([128, TPC, 128], bf16, tag="ext")
                        nc.gpsimd.dma_gather(
                            ext1[:], SDT[:],
                            dstl_t[:, ci1 * IC:(ci1 + 1) * IC],
                            CHUNK, CHUNK, 128)
                        nc.vector.tensor_copy(
                            sdall1[:, ci1 * TPC:(ci1 + 1) * TPC, :],
                            ext1[:, :, 8:16])
                        pre1_done += 1

            # ---- L1 edge phase + finalize (+ split AllGather) ----
            blk_ps = {}
            ag_done = 0
            pre_done = 0

            GR = NLOC // GAG

            def emit_ag(g):
                nc.gpsimd.collective_compute(
                    "AllGather", mybir.AluOpType.bypass,
                    replica_groups=[list(range(NC))],
                    ins=[agincs[g][:]],
                    outs=[agouts[g][:]])

            for ci in range(NCHUNK if upto != "A" else 0):
                # one-hot builds first: they have no gather dependency, so
                # DVE chews them while this chunk's gathers are in flight
                ohx_t = {}
                for tt in range(TPC):
                    t = ci * TPC + tt
                    if t >= TILES:
                        continue
                    ohx = ohp.tile([128, 128], bf16, tag="ohx")
                    nc.vector.tensor_scalar(
                        ohx[:], iota_t[:], dstoffT_t[:, t:t + 1], None,
                        Alu.is_equal)
                    ohx_t[tt] = ohx
                xrow = gx.tile([128, TPC, XW], bf16, tag="xrow")
                nc.gpsimd.dma_gather(
                    xrow[:], XPT[:], srcg_t[:, ci * IC:(ci + 1) * IC],
                    CHUNK, CHUNK, XW)
                # alpha: z = ss[src]+sd[dst], leaky, exp, *w
                z = sm.tile([128, TPC, 4], f32, tag="z")
                nc.vector.tensor_tensor(
                    z[:], xrow[:, :, 256:264].bitcast(f32),
                    sdall1[:, ci * TPC:(ci + 1) * TPC, :].bitcast(f32),
                    op=Alu.add)
                nc.vector.scalar_tensor_tensor(
                    z[:], in0=z[:], scalar=NEG, in1=z[:],
                    op0=Alu.mult, op1=Alu.max)
                ex = sm.tile([128, TPC, 4], f32, tag="ex")
                nc.scalar.activation(ex[:], z[:], Act.Exp)
                exw = sm.tile([128, TPC, 4], f32, tag="exw")
                wb = wT_t[:, ci * TPC:(ci + 1) * TPC]
                nc.vector.tensor_tensor(
                    exw[:], ex[:],
                    wb.rearrange("p (t o) -> p t o", o=1).to_broadcast(
                        [128, TPC, 4]), op=Alu.mult)
                # pair-packed copy of exw so the big scale op runs in 2x mode
                exw2 = sm.tile([128, TPC, 4, 2], bf16, tag="exw2")
                nc.vector.tensor_copy(
                    exw2[:], exw[:].rearrange("p t (h o) -> p t h o", o=1)
                    .to_broadcast([128, TPC, 4, 2]))
                xrh = xrow[:, :, 0:256].rearrange(
                    "p t (h k two) -> p t h k two", h=4, two=2)
                nc.vector.tensor_tensor(
                    xrh, xrh,
                    exw2[:].rearrange("p t h (o two) -> p t h o two", two=2)
                    .to_broadcast([128, TPC, 4, 32, 2]), op=Alu.mult)
                # unweighted ex -> hi/lo bf16 denominator cols 256:264
                nc.vector.tensor_copy(xrow[:, :, 256:260], ex[:])
                nc.vector.tensor_tensor(xrow[:, :, 260:264], ex[:],
                                        xrow[:, :, 256:260], op=Alu.subtract)

                for tt in range(TPC):
                    t = ci * TPC + tt
                    if t >= TILES:
                        continue
                    b = t // TPB
                    k = t % TPB
                    if k == 0:
                        blk_ps[b] = ps2.tile([128, 264], f32, tag="blk",
                                             name="blkps")
                    ps = blk_ps[b]
                    nc.tensor.matmul(
                        ps[:, 0:264], ohx_t[tt][:], xrow[:, tt, 0:264],
                        start=(k == 0), stop=(k == TPB - 1))
                    if k == TPB - 1:
                        # finalize block b -> h, then next-layer rows
                        den8 = sm.tile([128, 8], f32, tag="den8")
                        nc.vector.tensor_copy(den8[:], ps[:, 256:264])
                        den = sm.tile([128, 4], f32, tag="den")
                        nc.vector.tensor_tensor(den[:], den8[:, 0:4],
                                                den8[:, 4:8], op=Alu.add)
                        nc.vector.tensor_scalar_add(den[:], den[:], EPS)
                        rec = sm.tile([128, 4], f32, tag="rec")
                        nc.vector.reciprocal(rec[:], den[:])
                        hb = fin.tile([128, 256], f32, tag="hb")
                        for h in range(H):
                            nc.vector.scalar_tensor_tensor(
                                hb[:, h * 64:(h + 1) * 64],
                                in0=ps[:, h * 64:(h + 1) * 64],
                                scalar=rec[:, h:h + 1],
                                in1=b1b_t[:, h * 64:(h + 1) * 64],
                                op0=Alu.mult, op1=Alu.add)
                        # ELU: h = max(z,0) + exp(min(z,0)) - 1
                        zm = fin.tile([128, 256], f32, tag="zm")
                        nc.vector.tensor_scalar_min(zm[:], hb[:], 0.0)
                        ez = fin.tile([128, 256], f32, tag="ez")
                        nc.scalar.activation(ez[:], zm[:], Act.Exp)
                        nc.vector.scalar_tensor_tensor(
                            hb[:], in0=hb[:], scalar=0.0, in1=ez[:],
                            op0=Alu.max, op1=Alu.add)
                        nc.vector.tensor_scalar_add(hb[:], hb[:], -1.0)
                        # transpose h (2 x 128x128) and project
                        hTs = []
                        for half in range(2):
                            pst = ps1.tile([128, 128], f32, tag="pst")
                            nc.tensor.transpose(
                                pst[:], hb[:, half * 128:(half + 1) * 128],
                                ident_t[:])
                            hT = fin.tile([128, 128], bf16, tag=f"hT{half}")
                            nc.vector.tensor_copy(hT[:], pst[:])
                            hTs.append(hT)
                        psmulv = ps1.tile([128, 260], f32, tag="psmulv")
                        psmu = psmulv[:, 0:130]
                        pslv = psmulv[:, 130:260]
                        for kk in range(2):
                            nc.tensor.matmul(psmu, hTs[kk][:], wmue_t[kk][:],
                                             start=(kk == 0), stop=(kk == 1))
                            nc.tensor.matmul(pslv, hTs[kk][:], wlve_t[kk][:],
                                             start=(kk == 0), stop=(kk == 1))
                        # agin row: [xpmu | xplv | ssmu sdmu sslv sdlv (f32)]
                        xr2 = fin.tile([128, 264], bf16, tag="xr2")
                        nc.scalar.copy(xr2[:, 0:128], psmu[:, 0:128])
                        nc.scalar.copy(xr2[:, 128:256], pslv[:, 0:128])
                        ssv = xr2[:, 256:264].bitcast(f32)
                        nc.vector.tensor_copy(ssv[:, 0:2], psmu[:, 128:130])
                        nc.vector.tensor_copy(ssv[:, 2:4], pslv[:, 128:130])
                        nc.sync.dma_start(
                            agin[b * 128:(b + 1) * 128, 0:264], xr2[:])
                        gb = b // (BLOCKS // GAG)
                        rb = b % (BLOCKS // GAG)
                        nc.sync.dma_start(
                            agincs[gb][rb * 128:(rb + 1) * 128, :], xr2[:])
                        nc.sync.dma_start(
                            SD2T[b * 128:(b + 1) * 128, 0:8], xr2[:, 256:264])
                        del blk_ps[b]

                if upto in ("AG", "full"):
                    while ag_done < GAG and ci >= AG_EMIT[ag_done]:
                        emit_ag(ag_done)
                        ag_done += 1

            # ---- repack compact AG outputs into the 768B-stride table ----
            # (wait_until_ts keeps the scheduler from hoisting these into the
            # middle of L1 where their collective-wait would hold the queue)
            if upto in ("AG", "full"):
                for g in range(GAG):
                    nc.scalar.dma_start(
                        agall[:].rearrange("(c r) w -> c r w", r=NLOC)
                        [:, g * GR:(g + 1) * GR, 0:264],
                        agouts[g][:].rearrange("(c r) w -> c r w", r=GR))

            # ---- L2/3 dst-logit prefetches (overlap the AG tail) ----
            for ci in range(NCHUNK if upto == "full" else 0):
                ext2 = ge.tile([128, TPC, 128], bf16, tag="ext")
                nc.gpsimd.dma_gather(
                    ext2[:], SD2T[:], dstl_t[:, ci * IC:(ci + 1) * IC],
                    CHUNK, CHUNK, 128)
                nc.vector.tensor_copy(
                    sdall[:, ci * TPC:(ci + 1) * TPC, :], ext2[:, :, 0:8])

            # ---- L2/3 edge phase (mu and lv share gathers) ----
            blk2 = {}
            for ci in range(NCHUNK if upto == "full" else 0):
                oh2_t = {}
                for tt in range(TPC):
                    t = ci * TPC + tt
                    if t >= TILES:
                        continue
                    ohx = ohp.tile([128, 128], bf16, tag="ohx")
                    nc.vector.tensor_scalar(
                        ohx[:], iota_t[:], dstoffT_t[:, t:t + 1], None,
                        Alu.is_equal)
                    oh2_t[tt] = ohx
                xrow = gx.tile([128, TPC, XW], bf16, tag="xrow")
                nc.gpsimd.dma_gather(
                    xrow[:], agall[:], src2_t[:, ci * IC:(ci + 1) * IC],
                    CHUNK, CHUNK, XW)
                sl = slice(ci * TPC, (ci + 1) * TPC)
                z = sm.tile([128, TPC, 2], f32, tag="z2")
                nc.vector.tensor_tensor(
                    z[:],
                    xrow[:, :, 256:264].bitcast(f32)
                    .rearrange("p t (g s) -> p t g s", s=2)[:, :, :, 0],
                    sdall[:, sl, :].bitcast(f32)
                    .rearrange("p t (g s) -> p t g s", s=2)[:, :, :, 1],
                    op=Alu.add)
                nc.vector.scalar_tensor_tensor(
                    z[:], in0=z[:], scalar=NEG, in1=z[:],
                    op0=Alu.mult, op1=Alu.max)
                ex = sm.tile([128, TPC, 2], f32, tag="ex2")
                nc.scalar.activation(ex[:], z[:], Act.Exp)
                exw = sm.tile([128, TPC, 2], f32, tag="exw2")
                wb = wT_t[:, ci * TPC:(ci + 1) * TPC]
                nc.vector.tensor_tensor(
                    exw[:], ex[:],
                    wb.rearrange("p (t o) -> p t o", o=1).to_broadcast(
                        [128, TPC, 2]), op=Alu.mult)
                exw2 = sm.tile([128, TPC, 2, 2], bf16, tag="exw22")
                nc.vector.tensor_copy(
                    exw2[:], exw[:].rearrange("p t (g o) -> p t g o", o=1)
                    .to_broadcast([128, TPC, 2, 2]))
                xrg = xrow[:, :, 0:256].rearrange(
                    "p t (g k two) -> p t g k two", g=2, two=2)
                nc.vector.tensor_tensor(
                    xrg, xrg,
                    exw2[:].rearrange("p t g (o two) -> p t g o two", two=2)
                    .to_broadcast([128, TPC, 2, 64, 2]), op=Alu.mult)
                nc.vector.tensor_copy(xrow[:, :, 256:258], ex[:])
                nc.vector.tensor_tensor(xrow[:, :, 258:260], ex[:],
                                        xrow[:, :, 256:258], op=Alu.subtract)

                for tt in range(TPC):
                    t = ci * TPC + tt
                    if t >= TILES:
                        continue
                    b = t // TPB
                    k = t % TPB
                    if k == 0:
                        blk2[b] = ps2.tile([128, 260], f32, tag="blk",
                                           name="blk2ps")
                    ps2t = blk2[b]
                    nc.tensor.matmul(
                        ps2t[:, 0:260], oh2_t[tt][:], xrow[:, tt, 0:260],
                        start=(k == 0), stop=(k == TPB - 1))
                    if k == TPB - 1:
                        for li, (bias_t, outdr) in enumerate(
                                ((bmub_t, mu_out), (blvb_t, lv_out))):
                            den4 = sm.tile([128, 4], f32, tag="den4")
                            nc.vector.tensor_copy(den4[:], ps2t[:, 256:260])
                            den = sm.tile([128, 1], f32, tag="den2")
                            nc.vector.tensor_tensor(
                                den[:], den4[:, li:li + 1],
                                den4[:, 2 + li:3 + li], op=Alu.add)
                            nc.vector.tensor_scalar_add(den[:], den[:], EPS)
                            rec = sm.tile([128, 1], f32, tag="rec2")
                            nc.vector.reciprocal(rec[:], den[:])
                            ob = fin.tile([128, 128], f32, tag="ob")
                            nc.vector.scalar_tensor_tensor(
                                ob[:], in0=ps2t[:, li * 128:(li + 1) * 128],
                                scalar=rec[:, 0:1],
                                in1=bias_t[:], op0=Alu.mult, op1=Alu.add)
                            nc.sync.dma_start(
                                outdr[b * 128:(b + 1) * 128, :], ob[:])
                        del blk2[b]

    nc.compile()
    return nc


def _prep_inputs(x, edge_index, edge_weight, W1, att1, b1, Wmu, attmu, bmu,
                 Wlv, attlv, blv):
    import ml_dtypes
    bf = ml_dtypes.bfloat16

    src = np.asarray(edge_index[0], np.int64)
    dst = np.asarray(edge_index[1], np.int64)
    w = np.asarray(edge_weight, np.float32)
    x = np.asarray(x, np.float32)

    # fused weights
    att1 = np.asarray(att1, np.float32)          # [H, 2*C1]
    W1 = np.asarray(W1, np.float32)
    Wss1 = np.zeros((FIN, H), np.float32)
    Wsd1 = np.zeros((FIN, H), np.float32)
    for h in range(H):
        Wss1[:, h] = W1[:, h * C1:(h + 1) * C1] @ att1[h, C1:]
        Wsd1[:, h] = W1[:, h * C1:(h + 1) * C1] @ att1[h, :C1]
    w1e = np.concatenate([W1, Wss1, Wsd1], axis=1).astype(bf)   # [512, 264]

    attmu = np.asarray(attmu, np.float32).reshape(-1)
    attlv = np.asarray(attlv, np.float32).reshape(-1)
    Wmu = np.asarray(Wmu, np.float32)
    Wlv = np.asarray(Wlv, np.float32)
    wmue = np.concatenate(
        [Wmu, (Wmu @ attmu[LAT:])[:, None], (Wmu @ attmu[:LAT])[:, None]],
        axis=1).astype(bf)
    wlve = np.concatenate(
        [Wlv, (Wlv @ attlv[LAT:])[:, None], (Wlv @ attlv[:LAT])[:, None]],
        axis=1).astype(bf)

    xT_all = x.T.astype(bf)
    b1b = np.tile(np.asarray(b1, np.float32)[None, :], (128, 1))
    bmub = np.tile(np.asarray(bmu, np.float32)[None, :], (128, 1))
    blvb = np.tile(np.asarray(blv, np.float32)[None, :], (128, 1))
    iota = np.tile(np.arange(128, dtype=np.float32)[None, :],
                   (128, 1)).astype(bf)
    ident = np.eye(128, dtype=np.float32)

    # sort edges by dst, bucket per core, pad per 128-node block to TPB tiles
    order = np.argsort(dst, kind="stable")
    ssrc, sdst, sw = src[order], dst[order], w[order]
    core_of = sdst // NOWN
    in_maps = []
    for c in range(NC):
        m = core_of == c
        cs, cd, cw = ssrc[m], sdst[m] - c * NOWN, sw[m]
        blk = cd // 128
        e_src = np.zeros(EPAD, np.int64)
        e_dstloc = np.zeros(EPAD, np.int64)
        e_dstoff = np.full(EPAD, -1.0, np.float32)
        e_w = np.zeros(EPAD, np.float32)
        for b in range(BLOCKS):
            bm = blk == b
            nbe = int(bm.sum())
            if nbe > TPB * 128:
                raise RuntimeError(f"block overflow core {c} block {b}: {nbe}")
            o = b * TPB * 128
            e_src[o:o + nbe] = cs[bm]
            e_dstloc[o:o + nbe] = cd[bm]
            e_dstoff[o:o + nbe] = (cd[bm] - b * 128).astype(np.float32)
            e_w[o:o + nbe] = cw[bm]
        own = e_src // NOWN
        e_src2 = own * NLOC + (e_src - own * NOWN)
        # per-core node permutation: own dst nodes first (rows 0:2500)
        perm = np.concatenate([
            np.arange(c * NOWN, (c + 1) * NOWN),
            np.arange(0, c * NOWN),
            np.arange((c + 1) * NOWN, N)])
        inv = np.empty(N, np.int64)
        inv[perm] = np.arange(N)
        xTb_c = np.zeros((FIN, NPADA), bf)
        xTb_c[:, :N] = xT_all[:, perm]
        in_maps.append({
            "xTb": xTb_c, "w1e": w1e, "wmue": wmue, "wlve": wlve, "b1b": b1b,
            "bmub": bmub, "blvb": blvb, "iota": iota, "ident": ident,
            "identb": ident.astype(bf),
            "srcg": _wrap_idxs(inv[e_src]),
            "src2": _wrap_idxs(e_src2), "dstl": _wrap_idxs(e_dstloc),
            "dstoffT": _colmajor(e_dstoff),
            "wT": _colmajor(e_w),
        })
    return in_maps


def kernel(x, edge_index, edge_weight, W1, att1, b1, Wmu, attmu, bmu,
           Wlv, attlv, blv):
    from concourse.bass_utils import run_bass_kernel_spmd

    if "nc" not in _cache:
        _cache["nc"] = _build_module()
    nc = _cache["nc"]
    in_maps = _prep_inputs(x, edge_index, edge_weight, W1, att1, b1,
                           Wmu, attmu, bmu, Wlv, attlv, blv)
    r = run_bass_kernel_spmd(nc, in_maps, list(range(NC)))
    mu = np.zeros((N, LAT), np.float32)
    lv = np.zeros((N, LAT), np.float32)
    for c in range(NC):
        mu[c * NOWN:(c + 1) * NOWN] = r.results[c]["mu_out"][:NOWN]
        lv[c * NOWN:(c + 1) * NOWN] = r.results[c]["lv_out"][:NOWN]
    return (mu, lv)


# revision 10
# speedup vs baseline: 1.0305x; 1.0305x over previous
"""GAT encoder (4-head 256 concat + mu/logvar 128) on 8 trn2 cores — v2.

Strategy (dst-range node sharding):
 - Host sorts edges by dst, buckets per core (2500 dst nodes each), pads each
   128-node block to TPB tiles of 128 edges; 2048-edge gather chunks.
 - Phase A (per core, redundant): one fused bf16 matmul xTb @ [W1|Wss1|Wsd1]
   per 128-node tile -> XPT rows [xp bf16 0:256 | ss1 f32 | sd1 f32] (768B).
 - L1 edge phase: 768B dma_gather of XPT rows by src (payload+ss), 256B
   sub-row gather of XPT[256:384] by dst (sd); softmax coefs scaled into the
   payload in place; per 128-edge tile a one-hot bf16 matmul accumulates
   messages + ex hi/lo denominator columns into a per-block PSUM.
 - L1 finalize per block: normalize, +bias, ELU (bf16) -> h; PE-transpose and
   project with [Wmu|vmu|umu]/[Wlv|vlv|ulv]; rows [xpmu|xplv|logits f32] go to
   agin. AllGather is split into 4 chunks of 5 blocks, each issued as soon as
   its blocks are final so the collective overlaps the L1 tail.
 - L2/3 edge phase: 768B gather of agout rows by src, 256B sub-row gather of
   agin[256:384] by dst (prefetched into a compact sd table during the AG
   tail); same one-hot trick, mu and lv share one gather/matmul per tile.
Outputs (mu, logvar) assembled host-side from per-core slices.
"""

import numpy as np

# ---- problem constants (hardcoded per contract) ----
N = 20000
E = 320000
FIN = 512
HID = 256
LAT = 128
H = 4
C1 = 64
NEG = 0.2
EPS = 1e-16

NC = 8
NOWN = 2500          # dst nodes per core
BLOCKS = 20          # 128-node blocks per core
NLOC = BLOCKS * 128  # 2560
TPB = 18             # tiles (128 edges) per block (max real block = 2174 edges)
TILES = BLOCKS * TPB       # 360 real tiles
TPC = 8                   # tiles per gather chunk
CHUNK = TPC * 128
IC = CHUNK // 16           # idx table cols per chunk
NCHUNK = 45
TILES_PAD = NCHUNK * TPC   # 360
EPAD = TILES_PAD * 128     # 47104 edge slots per core
NPADA = 160 * 128          # 20480 padded global rows
XW = 384                   # XPT/agin/agout row width in bf16 slots (768B)
GAG = 4                    # AllGather split count (5 blocks each)
AG_EMIT = (12, 23, 34, 44)  # chunk after which AG_g is emitted (deps: 5,11,16,22)

_cache = {}


def _wrap_idxs(idx):
    n = idx.shape[0]
    t = np.zeros((128, n // 16), np.int16)
    w = idx.reshape(n // 16, 16).T.astype(np.int16)
    for g in range(8):
        t[g * 16:(g + 1) * 16, :] = w
    return t


def _colmajor(a):
    # per-edge array [EPAD] -> [128, TILES_PAD] tile-column layout
    return np.ascontiguousarray(a.reshape(TILES_PAD, 128).T)


def _build_module(upto="full"):
    import concourse.bacc as bacc
    import concourse.mybir as mybir
    import concourse.tile as tile

    f32 = mybir.dt.float32
    bf16 = mybir.dt.bfloat16
    i16 = mybir.dt.int16
    Alu = mybir.AluOpType
    Act = mybir.ActivationFunctionType

    nc = bacc.Bacc("TRN2", target_bir_lowering=False, num_devices=NC)

    # ---- inputs ----
    xTb = nc.dram_tensor("xTb", [FIN, NPADA], bf16, kind="ExternalInput")
    w1e = nc.dram_tensor("w1e", [FIN, 264], bf16, kind="ExternalInput")
    wmue = nc.dram_tensor("wmue", [HID, 130], bf16, kind="ExternalInput")
    wlve = nc.dram_tensor("wlve", [HID, 130], bf16, kind="ExternalInput")
    b1b = nc.dram_tensor("b1b", [128, 256], f32, kind="ExternalInput")
    bmub = nc.dram_tensor("bmub", [128, 128], f32, kind="ExternalInput")
    blvb = nc.dram_tensor("blvb", [128, 128], f32, kind="ExternalInput")
    iota = nc.dram_tensor("iota", [128, 128], bf16, kind="ExternalInput")
    ident = nc.dram_tensor("ident", [128, 128], f32, kind="ExternalInput")
    identb = nc.dram_tensor("identb", [128, 128], bf16, kind="ExternalInput")
    srcg = nc.dram_tensor("srcg", [128, EPAD // 16], i16, kind="ExternalInput")
    src2 = nc.dram_tensor("src2", [128, EPAD // 16], i16, kind="ExternalInput")
    dstl = nc.dram_tensor("dstl", [128, EPAD // 16], i16, kind="ExternalInput")
    dstoffT = nc.dram_tensor("dstoffT", [128, TILES_PAD], f32, kind="ExternalInput")
    wT = nc.dram_tensor("wT", [128, TILES_PAD], f32, kind="ExternalInput")

    mu_out = nc.dram_tensor("mu_out", [NLOC, LAT], f32, kind="ExternalOutput")
    lv_out = nc.dram_tensor("lv_out", [NLOC, LAT], f32, kind="ExternalOutput")

    with tile.TileContext(nc) as tc:
        with (
            tc.tile_pool(name="cst", bufs=1) as cst,
            tc.tile_pool(name="lw", bufs=3) as lw,
            tc.tile_pool(name="xa", bufs=3) as xa,
            tc.tile_pool(name="gx", bufs=3) as gx,
            tc.tile_pool(name="ge", bufs=3) as ge,
            tc.tile_pool(name="oh", bufs=20) as ohp,
            tc.tile_pool(name="sm", bufs=6) as sm,
            tc.tile_pool(name="fin", bufs=3) as fin,
            tc.tile_pool(name="ps2", bufs=3, space="PSUM") as ps2,
            tc.tile_pool(name="psa", bufs=2, space="PSUM") as psa,
            tc.tile_pool(name="ps1", bufs=1, space="PSUM") as ps1,
            tc.tile_pool(name="dr", bufs=1, space="DRAM") as dr,
        ):
            # internal DRAM tables (aginc is the contiguous collective
            # input; wide tables keep 768B rows for gathers)
            XPT = dr.tile([NPADA, XW], bf16, tag="XPT")
            SDT = dr.tile([NLOC, 128], bf16, tag="SDT")
            SD2T = dr.tile([NLOC, 128], bf16, tag="SD2T")
            agin = dr.tile([NLOC, XW], bf16, tag="agin")
            agincs = []
            for g in range(GAG):
                aginc_g = dr.tile([NLOC // GAG, 264], bf16,
                                  tag=f"aginc{g}", name=f"aginc{g}")
                agincs.append(aginc_g)
            agall = dr.tile([NC * NLOC, XW], bf16, tag="agall")
            agouts = []
            for g in range(GAG):
                agout_g = dr.tile([NC * (NLOC // GAG), 264], bf16,
                                  tag=f"agout{g}", name=f"agout{g}",
                                  addr_space="Shared")
                agouts.append(agout_g)

            # resident constants
            w1e_t = []
            for kk in range(4):
                t = cst.tile([128, 264], bf16, tag=f"w1e{kk}")
                nc.sync.dma_start(t[:], w1e[kk * 128:(kk + 1) * 128, :])
                w1e_t.append(t)
            wmue_t = []
            wlve_t = []
            for kk in range(2):
                t = cst.tile([128, 130], bf16, tag=f"wmue{kk}")
                nc.sync.dma_start(t[:], wmue[kk * 128:(kk + 1) * 128, :])
                wmue_t.append(t)
                t2 = cst.tile([128, 130], bf16, tag=f"wlve{kk}")
                nc.sync.dma_start(t2[:], wlve[kk * 128:(kk + 1) * 128, :])
                wlve_t.append(t2)
            b1b_t = cst.tile([128, 256], f32, tag="b1b")
            nc.sync.dma_start(b1b_t[:], b1b[:])
            bmub_t = cst.tile([128, 128], f32, tag="bmub")
            nc.sync.dma_start(bmub_t[:], bmub[:])
            blvb_t = cst.tile([128, 128], f32, tag="blvb")
            nc.sync.dma_start(blvb_t[:], blvb[:])
            iota_t = cst.tile([128, 128], bf16, tag="iota")
            nc.sync.dma_start(iota_t[:], iota[:])
            ident_t = cst.tile([128, 128], f32, tag="ident")
            nc.sync.dma_start(ident_t[:], ident[:])
            identb_t = cst.tile([128, 128], bf16, tag="identb")
            nc.sync.dma_start(identb_t[:], identb[:])
            srcg_t = cst.tile([128, EPAD // 16], i16, tag="srcg")
            nc.sync.dma_start(srcg_t[:], srcg[:])
            src2_t = cst.tile([128, EPAD // 16], i16, tag="src2")
            nc.sync.dma_start(src2_t[:], src2[:])
            dstl_t = cst.tile([128, EPAD // 16], i16, tag="dstl")
            nc.sync.dma_start(dstl_t[:], dstl[:])
            dstoffT_t = cst.tile([128, TILES_PAD], f32, tag="dstoffT")
            nc.sync.dma_start(dstoffT_t[:], dstoffT[:])
            wT_t = cst.tile([128, TILES_PAD], f32, tag="wT")
            nc.sync.dma_start(wT_t[:], wT[:])
            # compact per-edge dst logits for L2/3, filled during the AG tail
            sdall = cst.tile([128, TILES_PAD, 8], bf16, tag="sdall")
            # compact per-edge dst logits for L1, filled during phase A
            sdall1 = cst.tile([128, TILES_PAD, 8], bf16, tag="sdall1")

            # ---- phase A (replicated, own-first row permutation): one
            # fused matmul per 128-node tile -> XPT rows; own rows land in
            # groups 0-4 so L1 dst-logit gathers overlap later groups ----
            pre1_done = 0
            for g in range(NPADA // 512):
                lx = lw.tile([128, 4, 512], bf16, tag="lx")
                nc.sync.dma_start(
                    lx[:], xTb[:].rearrange("(kk p) (g n) -> p kk g n",
                                            p=128, n=512)[:, :, g, :])
                xps = xa.tile([128, 4, 272], bf16, tag="xps")
                for ti in range(4):
                    ps = psa.tile([128, 264], f32, tag="psA", name="psA")
                    for kk in range(4):
                        sl = slice(ti * 128, (ti + 1) * 128)
                        nc.tensor.matmul(ps[:], lx[:, kk, sl], w1e_t[kk][:],
                                         start=(kk == 0), stop=(kk == 3))
                    nc.scalar.copy(xps[:, ti, 0:256], ps[:, 0:256])
                    nc.vector.tensor_copy(
                        xps[:, ti, 256:272].bitcast(f32), ps[:, 256:264])
                nc.sync.dma_start(
                    XPT[:].rearrange("(g4 p) c -> p g4 c", p=128)
                    [:, 4 * g:4 * g + 4, 0:272], xps[:])
                if g < 5:
                    nc.sync.dma_start(
                        SDT[:].rearrange("(g4 p) c -> p g4 c", p=128)
                        [:, 4 * g:4 * g + 4, 0:16], xps[:, :, 256:272])
                if g >= 6 and upto != "A":
                    for _ in range(2):
                        if pre1_done >= NCHUNK:
                            break
                        ci1 = pre1_done
                        ext1 = ge.tile([128, TPC, 128], bf16, tag="ext")
                        nc.gpsimd.dma_gather(
                            ext1[:], SDT[:],
                            dstl_t[:, ci1 * IC:(ci1 + 1) * IC],
                            CHUNK, CHUNK, 128)
                        nc.vector.tensor_copy(
                            sdall1[:, ci1 * TPC:(ci1 + 1) * TPC, :],
                            ext1[:, :, 8:16])
                        pre1_done += 1

            # ---- L1 edge phase + finalize (+ split AllGather) ----
            blk_ps = {}
            ag_done = 0
            pre_done = 0

            GR = NLOC // GAG

            def emit_ag(g):
                nc.gpsimd.collective_compute(
                    "AllGather", mybir.AluOpType.bypass,
                    replica_groups=[list(range(NC))],
                    ins=[agincs[g][:]],
                    outs=[agouts[g][:]])

            for ci in range(NCHUNK if upto != "A" else 0):
                # one-hot builds first: they have no gather dependency, so
                # DVE chews them while this chunk's gathers are in flight
                ohx_t = {}
                for tt in range(TPC):
                    t = ci * TPC + tt
                    if t >= TILES:
                        continue
                    ohx = ohp.tile([128, 128], bf16, tag="ohx")
                    nc.vector.tensor_scalar(
                        ohx[:], iota_t[:], dstoffT_t[:, t:t + 1], None,
                        Alu.is_equal)
                    ohx_t[tt] = ohx
                xrow = gx.tile([128, TPC, XW], bf16, tag="xrow")
                nc.gpsimd.dma_gather(
                    xrow[:], XPT[:], srcg_t[:, ci * IC:(ci + 1) * IC],
                    CHUNK, CHUNK, XW)
                # alpha: z = ss[src]+sd[dst], leaky, exp, *w
                z = sm.tile([128, TPC, 4], f32, tag="z")
                nc.vector.tensor_tensor(
                    z[:], xrow[:, :, 256:264].bitcast(f32),
                    sdall1[:, ci * TPC:(ci + 1) * TPC, :].bitcast(f32),
                    op=Alu.add)
                nc.vector.scalar_tensor_tensor(
                    z[:], in0=z[:], scalar=NEG, in1=z[:],
                    op0=Alu.mult, op1=Alu.max)
                ex = sm.tile([128, TPC, 4], f32, tag="ex")
                nc.scalar.activation(ex[:], z[:], Act.Exp)
                exw = sm.tile([128, TPC, 4], f32, tag="exw")
                wb = wT_t[:, ci * TPC:(ci + 1) * TPC]
                nc.vector.tensor_tensor(
                    exw[:], ex[:],
                    wb.rearrange("p (t o) -> p t o", o=1).to_broadcast(
                        [128, TPC, 4]), op=Alu.mult)
                # pair-packed copy of exw so the big scale op runs in 2x mode
                exw2 = sm.tile([128, TPC, 4, 2], bf16, tag="exw2")
                nc.vector.tensor_copy(
                    exw2[:], exw[:].rearrange("p t (h o) -> p t h o", o=1)
                    .to_broadcast([128, TPC, 4, 2]))
                xrh = xrow[:, :, 0:256].rearrange(
                    "p t (h k two) -> p t h k two", h=4, two=2)
                nc.vector.tensor_tensor(
                    xrh, xrh,
                    exw2[:].rearrange("p t h (o two) -> p t h o two", two=2)
                    .to_broadcast([128, TPC, 4, 32, 2]), op=Alu.mult)
                # unweighted ex -> hi/lo bf16 denominator cols 256:264
                nc.vector.tensor_copy(xrow[:, :, 256:260], ex[:])
                nc.vector.tensor_tensor(xrow[:, :, 260:264], ex[:],
                                        xrow[:, :, 256:260], op=Alu.subtract)

                for tt in range(TPC):
                    t = ci * TPC + tt
                    if t >= TILES:
                        continue
                    b = t // TPB
                    k = t % TPB
                    if k == 0:
                        blk_ps[b] = ps2.tile([128, 264], f32, tag="blk",
                                             name="blkps")
                    ps = blk_ps[b]
                    nc.tensor.matmul(
                        ps[:, 0:264], ohx_t[tt][:], xrow[:, tt, 0:264],
                        start=(k == 0), stop=(k == TPB - 1))
                    if k == TPB - 1:
                        # finalize block b -> h, then next-layer rows
                        den8 = sm.tile([128, 8], f32, tag="den8")
                        nc.vector.tensor_copy(den8[:], ps[:, 256:264])
                        den = sm.tile([128, 4], f32, tag="den")
                        nc.vector.tensor_tensor(den[:], den8[:, 0:4],
                                                den8[:, 4:8], op=Alu.add)
                        nc.vector.tensor_scalar_add(den[:], den[:], EPS)
                        rec = sm.tile([128, 4], f32, tag="rec")
                        nc.vector.reciprocal(rec[:], den[:])
                        hb = fin.tile([128, 256], f32, tag="hb")
                        for h in range(H):
                            nc.vector.scalar_tensor_tensor(
                                hb[:, h * 64:(h + 1) * 64],
                                in0=ps[:, h * 64:(h + 1) * 64],
                                scalar=rec[:, h:h + 1],
                                in1=b1b_t[:, h * 64:(h + 1) * 64],
                                op0=Alu.mult, op1=Alu.add)
                        # ELU: h = max(z,0) + exp(min(z,0)) - 1
                        zm = fin.tile([128, 256], f32, tag="zm")
                        nc.vector.tensor_scalar_min(zm[:], hb[:], 0.0)
                        ez = fin.tile([128, 256], f32, tag="ez")
                        nc.scalar.activation(ez[:], zm[:], Act.Exp)
                        nc.vector.scalar_tensor_tensor(
                            hb[:], in0=hb[:], scalar=0.0, in1=ez[:],
                            op0=Alu.max, op1=Alu.add)
                        nc.vector.tensor_scalar_add(hb[:], hb[:], -1.0)
                        # transpose h (2 x 128x128) and project
                        hTs = []
                        for half in range(2):
                            pst = ps1.tile([128, 128], f32, tag="pst")
                            nc.tensor.transpose(
                                pst[:], hb[:, half * 128:(half + 1) * 128],
                                ident_t[:])
                            hT = fin.tile([128, 128], bf16, tag=f"hT{half}")
                            nc.vector.tensor_copy(hT[:], pst[:])
                            hTs.append(hT)
                        psmu_t = ps1.tile([128, 130], f32, tag="psmu")
                        pslv_t = ps1.tile([128, 130], f32, tag="pslv")
                        psmu = psmu_t[:]
                        pslv = pslv_t[:]
                        for kk in range(2):
                            nc.tensor.matmul(psmu, hTs[kk][:], wmue_t[kk][:],
                                             start=(kk == 0), stop=(kk == 1))
                            nc.tensor.matmul(pslv, hTs[kk][:], wlve_t[kk][:],
                                             start=(kk == 0), stop=(kk == 1))
                        # agin row: [xpmu | xplv | ssmu sdmu sslv sdlv (f32)]
                        xr2 = fin.tile([128, 264], bf16, tag="xr2")
                        nc.scalar.copy(xr2[:, 0:128], psmu[:, 0:128])
                        nc.scalar.copy(xr2[:, 128:256], pslv[:, 0:128])
                        ssv = xr2[:, 256:264].bitcast(f32)
                        nc.vector.tensor_copy(ssv[:, 0:2], psmu[:, 128:130])
                        nc.vector.tensor_copy(ssv[:, 2:4], pslv[:, 128:130])
                        nc.sync.dma_start(
                            agin[b * 128:(b + 1) * 128, 0:264], xr2[:])
                        gb = b // (BLOCKS // GAG)
                        rb = b % (BLOCKS // GAG)
                        nc.sync.dma_start(
                            agincs[gb][rb * 128:(rb + 1) * 128, :], xr2[:])
                        nc.sync.dma_start(
                            SD2T[b * 128:(b + 1) * 128, 0:8], xr2[:, 256:264])
                        del blk_ps[b]

                if upto in ("AG", "full"):
                    while ag_done < GAG and ci >= AG_EMIT[ag_done]:
                        emit_ag(ag_done)
                        ag_done += 1

            # ---- repack compact AG outputs into the 768B-stride table ----
            # (wait_until_ts keeps the scheduler from hoisting these into the
            # middle of L1 where their collective-wait would hold the queue)
            if upto in ("AG", "full"):
                for g in range(GAG):
                    nc.scalar.dma_start(
                        agall[:].rearrange("(c r) w -> c r w", r=NLOC)
                        [:, g * GR:(g + 1) * GR, 0:264],
                        agouts[g][:].rearrange("(c r) w -> c r w", r=GR))

            # ---- L2/3 dst-logit prefetches (overlap the AG tail) ----
            for ci in range(NCHUNK if upto == "full" else 0):
                ext2 = ge.tile([128, TPC, 128], bf16, tag="ext")
                nc.gpsimd.dma_gather(
                    ext2[:], SD2T[:], dstl_t[:, ci * IC:(ci + 1) * IC],
                    CHUNK, CHUNK, 128)
                nc.vector.tensor_copy(
                    sdall[:, ci * TPC:(ci + 1) * TPC, :], ext2[:, :, 0:8])

            # ---- L2/3 edge phase (mu and lv share gathers) ----
            blk2 = {}
            for ci in range(NCHUNK if upto == "full" else 0):
                oh2_t = {}
                for tt in range(TPC):
                    t = ci * TPC + tt
                    if t >= TILES:
                        continue
                    ohx = ohp.tile([128, 128], bf16, tag="ohx")
                    nc.vector.tensor_scalar(
                        ohx[:], iota_t[:], dstoffT_t[:, t:t + 1], None,
                        Alu.is_equal)
                    oh2_t[tt] = ohx
                xrow = gx.tile([128, TPC, XW], bf16, tag="xrow")
                nc.gpsimd.dma_gather(
                    xrow[:], agall[:], src2_t[:, ci * IC:(ci + 1) * IC],
                    CHUNK, CHUNK, XW)
                sl = slice(ci * TPC, (ci + 1) * TPC)
                z = sm.tile([128, TPC, 2], f32, tag="z2")
                nc.vector.tensor_tensor(
                    z[:],
                    xrow[:, :, 256:264].bitcast(f32)
                    .rearrange("p t (g s) -> p t g s", s=2)[:, :, :, 0],
                    sdall[:, sl, :].bitcast(f32)
                    .rearrange("p t (g s) -> p t g s", s=2)[:, :, :, 1],
                    op=Alu.add)
                nc.vector.scalar_tensor_tensor(
                    z[:], in0=z[:], scalar=NEG, in1=z[:],
                    op0=Alu.mult, op1=Alu.max)
                ex = sm.tile([128, TPC, 2], f32, tag="ex2")
                nc.scalar.activation(ex[:], z[:], Act.Exp)
                exw = sm.tile([128, TPC, 2], f32, tag="exw2")
                wb = wT_t[:, ci * TPC:(ci + 1) * TPC]
                nc.vector.tensor_tensor(
                    exw[:], ex[:],
                    wb.rearrange("p (t o) -> p t o", o=1).to_broadcast(
                        [128, TPC, 2]), op=Alu.mult)
                exw2 = sm.tile([128, TPC, 2, 2], bf16, tag="exw22")
                nc.vector.tensor_copy(
                    exw2[:], exw[:].rearrange("p t (g o) -> p t g o", o=1)
                    .to_broadcast([128, TPC, 2, 2]))
                xrg = xrow[:, :, 0:256].rearrange(
                    "p t (g k two) -> p t g k two", g=2, two=2)
                nc.vector.tensor_tensor(
                    xrg, xrg,
                    exw2[:].rearrange("p t g (o two) -> p t g o two", two=2)
                    .to_broadcast([128, TPC, 2, 64, 2]), op=Alu.mult)
                nc.vector.tensor_copy(xrow[:, :, 256:258], ex[:])
                nc.vector.tensor_tensor(xrow[:, :, 258:260], ex[:],
                                        xrow[:, :, 256:258], op=Alu.subtract)

                for tt in range(TPC):
                    t = ci * TPC + tt
                    if t >= TILES:
                        continue
                    b = t // TPB
                    k = t % TPB
                    if k == 0:
                        blk2[b] = ps2.tile([128, 260], f32, tag="blk",
                                           name="blk2ps")
                    ps2t = blk2[b]
                    nc.tensor.matmul(
                        ps2t[:, 0:260], oh2_t[tt][:], xrow[:, tt, 0:260],
                        start=(k == 0), stop=(k == TPB - 1))
                    if k == TPB - 1:
                        for li, (bias_t, outdr) in enumerate(
                                ((bmub_t, mu_out), (blvb_t, lv_out))):
                            den4 = sm.tile([128, 4], f32, tag="den4")
                            nc.vector.tensor_copy(den4[:], ps2t[:, 256:260])
                            den = sm.tile([128, 1], f32, tag="den2")
                            nc.vector.tensor_tensor(
                                den[:], den4[:, li:li + 1],
                                den4[:, 2 + li:3 + li], op=Alu.add)
                            nc.vector.tensor_scalar_add(den[:], den[:], EPS)
                            rec = sm.tile([128, 1], f32, tag="rec2")
                            nc.vector.reciprocal(rec[:], den[:])
                            ob = fin.tile([128, 128], f32, tag="ob")
                            nc.vector.scalar_tensor_tensor(
                                ob[:], in0=ps2t[:, li * 128:(li + 1) * 128],
                                scalar=rec[:, 0:1],
                                in1=bias_t[:], op0=Alu.mult, op1=Alu.add)
                            nc.sync.dma_start(
                                outdr[b * 128:(b + 1) * 128, :], ob[:])
                        del blk2[b]

    nc.compile()
    return nc


def _prep_inputs(x, edge_index, edge_weight, W1, att1, b1, Wmu, attmu, bmu,
                 Wlv, attlv, blv):
    import ml_dtypes
    bf = ml_dtypes.bfloat16

    src = np.asarray(edge_index[0], np.int64)
    dst = np.asarray(edge_index[1], np.int64)
    w = np.asarray(edge_weight, np.float32)
    x = np.asarray(x, np.float32)

    # fused weights
    att1 = np.asarray(att1, np.float32)          # [H, 2*C1]
    W1 = np.asarray(W1, np.float32)
    Wss1 = np.zeros((FIN, H), np.float32)
    Wsd1 = np.zeros((FIN, H), np.float32)
    for h in range(H):
        Wss1[:, h] = W1[:, h * C1:(h + 1) * C1] @ att1[h, C1:]
        Wsd1[:, h] = W1[:, h * C1:(h + 1) * C1] @ att1[h, :C1]
    w1e = np.concatenate([W1, Wss1, Wsd1], axis=1).astype(bf)   # [512, 264]

    attmu = np.asarray(attmu, np.float32).reshape(-1)
    attlv = np.asarray(attlv, np.float32).reshape(-1)
    Wmu = np.asarray(Wmu, np.float32)
    Wlv = np.asarray(Wlv, np.float32)
    wmue = np.concatenate(
        [Wmu, (Wmu @ attmu[LAT:])[:, None], (Wmu @ attmu[:LAT])[:, None]],
        axis=1).astype(bf)
    wlve = np.concatenate(
        [Wlv, (Wlv @ attlv[LAT:])[:, None], (Wlv @ attlv[:LAT])[:, None]],
        axis=1).astype(bf)

    xT_all = x.T.astype(bf)
    b1b = np.tile(np.asarray(b1, np.float32)[None, :], (128, 1))
    bmub = np.tile(np.asarray(bmu, np.float32)[None, :], (128, 1))
    blvb = np.tile(np.asarray(blv, np.float32)[None, :], (128, 1))
    iota = np.tile(np.arange(128, dtype=np.float32)[None, :],
                   (128, 1)).astype(bf)
    ident = np.eye(128, dtype=np.float32)

    # sort edges by dst, bucket per core, pad per 128-node block to TPB tiles
    order = np.argsort(dst, kind="stable")
    ssrc, sdst, sw = src[order], dst[order], w[order]
    core_of = sdst // NOWN
    in_maps = []
    for c in range(NC):
        m = core_of == c
        cs, cd, cw = ssrc[m], sdst[m] - c * NOWN, sw[m]
        blk = cd // 128
        e_src = np.zeros(EPAD, np.int64)
        e_dstloc = np.zeros(EPAD, np.int64)
        e_dstoff = np.full(EPAD, -1.0, np.float32)
        e_w = np.zeros(EPAD, np.float32)
        for b in range(BLOCKS):
            bm = blk == b
            nbe = int(bm.sum())
            if nbe > TPB * 128:
                raise RuntimeError(f"block overflow core {c} block {b}: {nbe}")
            o = b * TPB * 128
            e_src[o:o + nbe] = cs[bm]
            e_dstloc[o:o + nbe] = cd[bm]
            e_dstoff[o:o + nbe] = (cd[bm] - b * 128).astype(np.float32)
            e_w[o:o + nbe] = cw[bm]
        own = e_src // NOWN
        e_src2 = own * NLOC + (e_src - own * NOWN)
        # per-core node permutation: own dst nodes first (rows 0:2500)
        perm = np.concatenate([
            np.arange(c * NOWN, (c + 1) * NOWN),
            np.arange(0, c * NOWN),
            np.arange((c + 1) * NOWN, N)])
        inv = np.empty(N, np.int64)
        inv[perm] = np.arange(N)
        xTb_c = np.zeros((FIN, NPADA), bf)
        xTb_c[:, :N] = xT_all[:, perm]
        in_maps.append({
            "xTb": xTb_c, "w1e": w1e, "wmue": wmue, "wlve": wlve, "b1b": b1b,
            "bmub": bmub, "blvb": blvb, "iota": iota, "ident": ident,
            "identb": ident.astype(bf),
            "srcg": _wrap_idxs(inv[e_src]),
            "src2": _wrap_idxs(e_src2), "dstl": _wrap_idxs(e_dstloc),
            "dstoffT": _colmajor(e_dstoff),
            "wT": _colmajor(e_w),
        })
    return in_maps


def kernel(x, edge_index, edge_weight, W1, att1, b1, Wmu, attmu, bmu,
           Wlv, attlv, blv):
    from concourse.bass_utils import run_bass_kernel_spmd

    if "nc" not in _cache:
        _cache["nc"] = _build_module()
    nc = _cache["nc"]
    in_maps = _prep_inputs(x, edge_index, edge_weight, W1, att1, b1,
                           Wmu, attmu, bmu, Wlv, attlv, blv)
    r = run_bass_kernel_spmd(nc, in_maps, list(range(NC)))
    mu = np.zeros((N, LAT), np.float32)
    lv = np.zeros((N, LAT), np.float32)
    for c in range(NC):
        mu[c * NOWN:(c + 1) * NOWN] = r.results[c]["mu_out"][:NOWN]
        lv[c * NOWN:(c + 1) * NOWN] = r.results[c]["lv_out"][:NOWN]
    return (mu, lv)


# revision 11
# speedup vs baseline: 1.1407x; 1.1069x over previous
"""GAT encoder (4-head 256 concat + mu/logvar 128) on 8 trn2 cores — v2.

Strategy (dst-range node sharding):
 - Host sorts edges by dst, buckets per core (2500 dst nodes each), pads each
   128-node block to TPB tiles of 128 edges; 2048-edge gather chunks.
 - Phase A (per core, redundant): one fused bf16 matmul xTb @ [W1|Wss1|Wsd1]
   per 128-node tile -> XPT rows [xp bf16 0:256 | ss1 f32 | sd1 f32] (768B).
 - L1 edge phase: 768B dma_gather of XPT rows by src (payload+ss), 256B
   sub-row gather of XPT[256:384] by dst (sd); softmax coefs scaled into the
   payload in place; per 128-edge tile a one-hot bf16 matmul accumulates
   messages + ex hi/lo denominator columns into a per-block PSUM.
 - L1 finalize per block: normalize, +bias, ELU (bf16) -> h; PE-transpose and
   project with [Wmu|vmu|umu]/[Wlv|vlv|ulv]; rows [xpmu|xplv|logits f32] go to
   agin. AllGather is split into 4 chunks of 5 blocks, each issued as soon as
   its blocks are final so the collective overlaps the L1 tail.
 - L2/3 edge phase: 768B gather of agout rows by src, 256B sub-row gather of
   agin[256:384] by dst (prefetched into a compact sd table during the AG
   tail); same one-hot trick, mu and lv share one gather/matmul per tile.
Outputs (mu, logvar) assembled host-side from per-core slices.
"""

import numpy as np

# ---- problem constants (hardcoded per contract) ----
N = 20000
E = 320000
FIN = 512
HID = 256
LAT = 128
H = 4
C1 = 64
NEG = 0.2
EPS = 1e-16

NC = 8
NOWN = 2500          # dst nodes per core
BLOCKS = 20          # 128-node blocks per core
NLOC = BLOCKS * 128  # 2560
TPB = 18             # tiles (128 edges) per block (max real block = 2174 edges)
TILES = BLOCKS * TPB       # 360 real tiles
TPC = 8                   # tiles per gather chunk
CHUNK = TPC * 128
IC = CHUNK // 16           # idx table cols per chunk
NCHUNK = 45
TILES_PAD = NCHUNK * TPC   # 360
EPAD = TILES_PAD * 128     # 47104 edge slots per core
NPADA = 160 * 128          # 20480 padded global rows
XW = 384                   # XPT/agin/agout row width in bf16 slots (768B)
GAG = 4                    # AllGather split count (5 blocks each)
AG_EMIT = (13, 24, 35, 44)  # chunk after which AG_g is emitted (deps: 5,11,16,22)

_cache = {}


def _wrap_idxs(idx):
    n = idx.shape[0]
    t = np.zeros((128, n // 16), np.int16)
    w = idx.reshape(n // 16, 16).T.astype(np.int16)
    for g in range(8):
        t[g * 16:(g + 1) * 16, :] = w
    return t


def _colmajor(a):
    # per-edge array [EPAD] -> [128, TILES_PAD] tile-column layout
    return np.ascontiguousarray(a.reshape(TILES_PAD, 128).T)


def _build_module(upto="full"):
    import concourse.bacc as bacc
    import concourse.mybir as mybir
    import concourse.tile as tile

    f32 = mybir.dt.float32
    bf16 = mybir.dt.bfloat16
    i16 = mybir.dt.int16
    Alu = mybir.AluOpType
    Act = mybir.ActivationFunctionType

    nc = bacc.Bacc("TRN2", target_bir_lowering=False, num_devices=NC)

    # ---- inputs ----
    xTb = nc.dram_tensor("xTb", [FIN, NPADA], bf16, kind="ExternalInput")
    w1e = nc.dram_tensor("w1e", [FIN, 264], bf16, kind="ExternalInput")
    wmue = nc.dram_tensor("wmue", [HID, 130], bf16, kind="ExternalInput")
    wlve = nc.dram_tensor("wlve", [HID, 130], bf16, kind="ExternalInput")
    b1b = nc.dram_tensor("b1b", [128, 256], f32, kind="ExternalInput")
    bmub = nc.dram_tensor("bmub", [128, 128], f32, kind="ExternalInput")
    blvb = nc.dram_tensor("blvb", [128, 128], f32, kind="ExternalInput")
    iota = nc.dram_tensor("iota", [128, 128], bf16, kind="ExternalInput")
    ident = nc.dram_tensor("ident", [128, 128], f32, kind="ExternalInput")
    identb = nc.dram_tensor("identb", [128, 128], bf16, kind="ExternalInput")
    srcg = nc.dram_tensor("srcg", [128, EPAD // 16], i16, kind="ExternalInput")
    src2 = nc.dram_tensor("src2", [128, EPAD // 16], i16, kind="ExternalInput")
    dstl = nc.dram_tensor("dstl", [128, EPAD // 16], i16, kind="ExternalInput")
    dstoffT = nc.dram_tensor("dstoffT", [128, TILES_PAD], f32, kind="ExternalInput")
    wT = nc.dram_tensor("wT", [128, TILES_PAD], f32, kind="ExternalInput")

    mu_out = nc.dram_tensor("mu_out", [NLOC, LAT], f32, kind="ExternalOutput")
    lv_out = nc.dram_tensor("lv_out", [NLOC, LAT], f32, kind="ExternalOutput")

    with tile.TileContext(nc) as tc:
        with (
            tc.tile_pool(name="cst", bufs=1) as cst,
            tc.tile_pool(name="lw", bufs=3) as lw,
            tc.tile_pool(name="xa", bufs=3) as xa,
            tc.tile_pool(name="gx", bufs=3) as gx,
            tc.tile_pool(name="ge", bufs=3) as ge,
            tc.tile_pool(name="oh", bufs=20) as ohp,
            tc.tile_pool(name="sm", bufs=6) as sm,
            tc.tile_pool(name="fin", bufs=3) as fin,
            tc.tile_pool(name="ps2", bufs=3, space="PSUM") as ps2,
            tc.tile_pool(name="psa", bufs=2, space="PSUM") as psa,
            tc.tile_pool(name="ps1", bufs=1, space="PSUM") as ps1,
            tc.tile_pool(name="dr", bufs=1, space="DRAM") as dr,
        ):
            # internal DRAM tables (aginc is the contiguous collective
            # input; wide tables keep 768B rows for gathers)
            XPT = dr.tile([NPADA, XW], bf16, tag="XPT")
            SDT = dr.tile([NLOC, 128], bf16, tag="SDT")
            SD2T = dr.tile([NLOC, 128], bf16, tag="SD2T")
            agin = dr.tile([NLOC, XW], bf16, tag="agin")
            agincs = []
            for g in range(GAG):
                aginc_g = dr.tile([NLOC // GAG, 264], bf16,
                                  tag=f"aginc{g}", name=f"aginc{g}")
                agincs.append(aginc_g)
            agall = dr.tile([NC * NLOC, XW], bf16, tag="agall")
            agouts = []
            for g in range(GAG):
                agout_g = dr.tile([NC * (NLOC // GAG), 264], bf16,
                                  tag=f"agout{g}", name=f"agout{g}",
                                  addr_space="Shared")
                agouts.append(agout_g)

            # resident constants
            w1e_t = []
            for kk in range(4):
                t = cst.tile([128, 264], bf16, tag=f"w1e{kk}")
                nc.sync.dma_start(t[:], w1e[kk * 128:(kk + 1) * 128, :])
                w1e_t.append(t)
            wmue_t = []
            wlve_t = []
            for kk in range(2):
                t = cst.tile([128, 130], bf16, tag=f"wmue{kk}")
                nc.sync.dma_start(t[:], wmue[kk * 128:(kk + 1) * 128, :])
                wmue_t.append(t)
                t2 = cst.tile([128, 130], bf16, tag=f"wlve{kk}")
                nc.sync.dma_start(t2[:], wlve[kk * 128:(kk + 1) * 128, :])
                wlve_t.append(t2)
            b1b_t = cst.tile([128, 256], f32, tag="b1b")
            nc.sync.dma_start(b1b_t[:], b1b[:])
            bmub_t = cst.tile([128, 128], f32, tag="bmub")
            nc.sync.dma_start(bmub_t[:], bmub[:])
            blvb_t = cst.tile([128, 128], f32, tag="blvb")
            nc.sync.dma_start(blvb_t[:], blvb[:])
            iota_t = cst.tile([128, 128], bf16, tag="iota")
            nc.sync.dma_start(iota_t[:], iota[:])
            ident_t = cst.tile([128, 128], f32, tag="ident")
            nc.sync.dma_start(ident_t[:], ident[:])
            identb_t = cst.tile([128, 128], bf16, tag="identb")
            nc.sync.dma_start(identb_t[:], identb[:])
            srcg_t = cst.tile([128, EPAD // 16], i16, tag="srcg")
            nc.sync.dma_start(srcg_t[:], srcg[:])
            src2_t = cst.tile([128, EPAD // 16], i16, tag="src2")
            nc.sync.dma_start(src2_t[:], src2[:])
            dstl_t = cst.tile([128, EPAD // 16], i16, tag="dstl")
            nc.sync.dma_start(dstl_t[:], dstl[:])
            dstoffT_t = cst.tile([128, TILES_PAD], f32, tag="dstoffT")
            nc.sync.dma_start(dstoffT_t[:], dstoffT[:])
            wT_t = cst.tile([128, TILES_PAD], f32, tag="wT")
            nc.sync.dma_start(wT_t[:], wT[:])
            # compact per-edge dst logits for L2/3, filled during the AG tail
            sdall = cst.tile([128, TILES_PAD, 8], bf16, tag="sdall")
            # compact per-edge dst logits for L1, filled during phase A
            sdall1 = cst.tile([128, TILES_PAD, 8], bf16, tag="sdall1")

            # ---- phase A (replicated, own-first row permutation): one
            # fused matmul per 128-node tile -> XPT rows; own rows land in
            # groups 0-4 so L1 dst-logit gathers overlap later groups ----
            pre1_done = 0
            for g in range(NPADA // 512):
                lx = lw.tile([128, 4, 512], bf16, tag="lx")
                nc.sync.dma_start(
                    lx[:], xTb[:].rearrange("(kk p) (g n) -> p kk g n",
                                            p=128, n=512)[:, :, g, :])
                xps = xa.tile([128, 4, 272], bf16, tag="xps")
                for ti in range(4):
                    ps = psa.tile([128, 264], f32, tag="psA", name="psA")
                    for kk in range(4):
                        sl = slice(ti * 128, (ti + 1) * 128)
                        nc.tensor.matmul(ps[:], lx[:, kk, sl], w1e_t[kk][:],
                                         start=(kk == 0), stop=(kk == 3))
                    nc.scalar.copy(xps[:, ti, 0:256], ps[:, 0:256])
                    nc.vector.tensor_copy(
                        xps[:, ti, 256:272].bitcast(f32), ps[:, 256:264])
                nc.sync.dma_start(
                    XPT[:].rearrange("(g4 p) c -> p g4 c", p=128)
                    [:, 4 * g:4 * g + 4, 0:272], xps[:])
                if g < 5:
                    nc.sync.dma_start(
                        SDT[:].rearrange("(g4 p) c -> p g4 c", p=128)
                        [:, 4 * g:4 * g + 4, 0:16], xps[:, :, 256:272])
                if g >= 6 and upto != "A":
                    for _ in range(2):
                        if pre1_done >= NCHUNK:
                            break
                        ci1 = pre1_done
                        ext1 = ge.tile([128, TPC, 128], bf16, tag="ext")
                        nc.gpsimd.dma_gather(
                            ext1[:], SDT[:],
                            dstl_t[:, ci1 * IC:(ci1 + 1) * IC],
                            CHUNK, CHUNK, 128)
                        nc.vector.tensor_copy(
                            sdall1[:, ci1 * TPC:(ci1 + 1) * TPC, :],
                            ext1[:, :, 8:16])
                        pre1_done += 1

            # ---- L1 edge phase + finalize (+ split AllGather) ----
            blk_ps = {}
            ag_done = 0
            pre_done = 0

            GR = NLOC // GAG

            def emit_ag(g):
                nc.gpsimd.collective_compute(
                    "AllGather", mybir.AluOpType.bypass,
                    replica_groups=[list(range(NC))],
                    ins=[agincs[g][:]],
                    outs=[agouts[g][:]])

            for ci in range(NCHUNK if upto != "A" else 0):
                # one-hot builds first: they have no gather dependency, so
                # DVE chews them while this chunk's gathers are in flight
                ohx_t = {}
                for tt in range(TPC):
                    t = ci * TPC + tt
                    if t >= TILES:
                        continue
                    ohx = ohp.tile([128, 128], bf16, tag="ohx")
                    nc.vector.tensor_scalar(
                        ohx[:], iota_t[:], dstoffT_t[:, t:t + 1], None,
                        Alu.is_equal)
                    ohx_t[tt] = ohx
                xrow = gx.tile([128, TPC, XW], bf16, tag="xrow")
                nc.gpsimd.dma_gather(
                    xrow[:], XPT[:], srcg_t[:, ci * IC:(ci + 1) * IC],
                    CHUNK, CHUNK, XW)
                # alpha: z = ss[src]+sd[dst], leaky, exp, *w
                z = sm.tile([128, TPC, 4], f32, tag="z")
                nc.vector.tensor_tensor(
                    z[:], xrow[:, :, 256:264].bitcast(f32),
                    sdall1[:, ci * TPC:(ci + 1) * TPC, :].bitcast(f32),
                    op=Alu.add)
                nc.vector.scalar_tensor_tensor(
                    z[:], in0=z[:], scalar=NEG, in1=z[:],
                    op0=Alu.mult, op1=Alu.max)
                ex = sm.tile([128, TPC, 4], f32, tag="ex")
                nc.scalar.activation(ex[:], z[:], Act.Exp)
                exw = sm.tile([128, TPC, 4], f32, tag="exw")
                wb = wT_t[:, ci * TPC:(ci + 1) * TPC]
                nc.vector.tensor_tensor(
                    exw[:], ex[:],
                    wb.rearrange("p (t o) -> p t o", o=1).to_broadcast(
                        [128, TPC, 4]), op=Alu.mult)
                # pair-packed copy of exw so the big scale op runs in 2x mode
                exw2 = sm.tile([128, TPC, 4, 2], bf16, tag="exw2")
                nc.vector.tensor_copy(
                    exw2[:], exw[:].rearrange("p t (h o) -> p t h o", o=1)
                    .to_broadcast([128, TPC, 4, 2]))
                xrh = xrow[:, :, 0:256].rearrange(
                    "p t (h k two) -> p t h k two", h=4, two=2)
                nc.vector.tensor_tensor(
                    xrh, xrh,
                    exw2[:].rearrange("p t h (o two) -> p t h o two", two=2)
                    .to_broadcast([128, TPC, 4, 32, 2]), op=Alu.mult)
                # unweighted ex -> hi/lo bf16 denominator cols 256:264
                nc.vector.tensor_copy(xrow[:, :, 256:260], ex[:])
                nc.vector.tensor_tensor(xrow[:, :, 260:264], ex[:],
                                        xrow[:, :, 256:260], op=Alu.subtract)

                for tt in range(TPC):
                    t = ci * TPC + tt
                    if t >= TILES:
                        continue
                    b = t // TPB
                    k = t % TPB
                    if k == 0:
                        blk_ps[b] = ps2.tile([128, 264], f32, tag="blk",
                                             name="blkps")
                    ps = blk_ps[b]
                    nc.tensor.matmul(
                        ps[:, 0:264], ohx_t[tt][:], xrow[:, tt, 0:264],
                        start=(k == 0), stop=(k == TPB - 1))
                    if k == TPB - 1:
                        # finalize block b -> h, then next-layer rows
                        den8 = sm.tile([128, 8], f32, tag="den8")
                        nc.vector.tensor_copy(den8[:], ps[:, 256:264])
                        den = sm.tile([128, 4], f32, tag="den")
                        nc.vector.tensor_tensor(den[:], den8[:, 0:4],
                                                den8[:, 4:8], op=Alu.add)
                        nc.vector.tensor_scalar_add(den[:], den[:], EPS)
                        rec = sm.tile([128, 4], f32, tag="rec")
                        nc.vector.reciprocal(rec[:], den[:])
                        hb = fin.tile([128, 256], f32, tag="hb")
                        for h in range(H):
                            nc.vector.scalar_tensor_tensor(
                                hb[:, h * 64:(h + 1) * 64],
                                in0=ps[:, h * 64:(h + 1) * 64],
                                scalar=rec[:, h:h + 1],
                                in1=b1b_t[:, h * 64:(h + 1) * 64],
                                op0=Alu.mult, op1=Alu.add)
                        # ELU: h = max(z,0) + exp(min(z,0)) - 1
                        zm = fin.tile([128, 256], f32, tag="zm")
                        nc.vector.tensor_scalar_min(zm[:], hb[:], 0.0)
                        ez = fin.tile([128, 256], f32, tag="ez")
                        nc.scalar.activation(ez[:], zm[:], Act.Exp)
                        nc.vector.scalar_tensor_tensor(
                            hb[:], in0=hb[:], scalar=0.0, in1=ez[:],
                            op0=Alu.max, op1=Alu.add)
                        nc.vector.tensor_scalar_add(hb[:], hb[:], -1.0)
                        # transpose h (2 x 128x128) and project
                        hTs = []
                        for half in range(2):
                            pst = ps1.tile([128, 128], f32, tag="pst")
                            nc.tensor.transpose(
                                pst[:], hb[:, half * 128:(half + 1) * 128],
                                ident_t[:])
                            hT = fin.tile([128, 128], bf16, tag=f"hT{half}")
                            nc.vector.tensor_copy(hT[:], pst[:])
                            hTs.append(hT)
                        psmu_t = ps1.tile([128, 130], f32, tag="psmu")
                        pslv_t = ps1.tile([128, 130], f32, tag="pslv")
                        psmu = psmu_t[:]
                        pslv = pslv_t[:]
                        for kk in range(2):
                            nc.tensor.matmul(psmu, hTs[kk][:], wmue_t[kk][:],
                                             start=(kk == 0), stop=(kk == 1))
                            nc.tensor.matmul(pslv, hTs[kk][:], wlve_t[kk][:],
                                             start=(kk == 0), stop=(kk == 1))
                        # agin row: [xpmu | xplv | ssmu sdmu sslv sdlv (f32)]
                        xr2 = fin.tile([128, 264], bf16, tag="xr2")
                        nc.scalar.copy(xr2[:, 0:128], psmu[:, 0:128])
                        nc.scalar.copy(xr2[:, 128:256], pslv[:, 0:128])
                        ssv = xr2[:, 256:264].bitcast(f32)
                        nc.vector.tensor_copy(ssv[:, 0:2], psmu[:, 128:130])
                        nc.vector.tensor_copy(ssv[:, 2:4], pslv[:, 128:130])
                        nc.sync.dma_start(
                            agin[b * 128:(b + 1) * 128, 0:264], xr2[:])
                        gb = b // (BLOCKS // GAG)
                        rb = b % (BLOCKS // GAG)
                        nc.sync.dma_start(
                            agincs[gb][rb * 128:(rb + 1) * 128, :], xr2[:])
                        nc.sync.dma_start(
                            SD2T[b * 128:(b + 1) * 128, 0:8], xr2[:, 256:264])
                        del blk_ps[b]

                if upto in ("AG", "full"):
                    while ag_done < GAG and ci >= AG_EMIT[ag_done]:
                        emit_ag(ag_done)
                        ag_done += 1
                # L2/3 dst-logit prefetches whose blocks are final (bounded
                # row range keeps the Pool-queue hold near zero)
                if upto == "full":
                    fin_blocks = ((ci - 2) * TPC + TPC - 1) // TPB if ci > 1 else -1
                    while (pre_done < NCHUNK
                           and (pre_done * TPC + TPC - 1) // TPB < fin_blocks):
                        ci2 = pre_done
                        hib = min((ci2 * TPC + TPC - 1) // TPB, BLOCKS - 1)
                        ext2 = ge.tile([128, TPC, 128], bf16, tag="ext")
                        nc.gpsimd.dma_gather(
                            ext2[:], SD2T[0:(hib + 1) * 128, :],
                            dstl_t[:, ci2 * IC:(ci2 + 1) * IC],
                            CHUNK, CHUNK, 128)
                        nc.vector.tensor_copy(
                            sdall[:, ci2 * TPC:(ci2 + 1) * TPC, :],
                            ext2[:, :, 0:8])
                        pre_done += 1

            # ---- repack compact AG outputs into the 768B-stride table ----
            # (wait_until_ts keeps the scheduler from hoisting these into the
            # middle of L1 where their collective-wait would hold the queue)
            if upto in ("AG", "full"):
                for g in range(GAG):
                    rp = nc.scalar.dma_start(
                        agall[:].rearrange("(c r) w -> c r w", r=NLOC)
                        [:, g * GR:(g + 1) * GR, 0:264],
                        agouts[g][:].rearrange("(c r) w -> c r w", r=GR))
                    rp.ins.bass_wait_until_ts = REPACK_TS[g]

            # ---- remaining L2/3 dst-logit prefetches ----
            for ci in range(pre_done, NCHUNK if upto == "full" else 0):
                ext2 = ge.tile([128, TPC, 128], bf16, tag="ext")
                nc.gpsimd.dma_gather(
                    ext2[:], SD2T[:], dstl_t[:, ci * IC:(ci + 1) * IC],
                    CHUNK, CHUNK, 128)
                nc.vector.tensor_copy(
                    sdall[:, ci * TPC:(ci + 1) * TPC, :], ext2[:, :, 0:8])

            # ---- L2/3 edge phase (mu and lv share gathers) ----
            blk2 = {}
            for ci in range(NCHUNK if upto == "full" else 0):
                oh2_t = {}
                for tt in range(TPC):
                    t = ci * TPC + tt
                    if t >= TILES:
                        continue
                    ohx = ohp.tile([128, 128], bf16, tag="ohx")
                    nc.vector.tensor_scalar(
                        ohx[:], iota_t[:], dstoffT_t[:, t:t + 1], None,
                        Alu.is_equal)
                    oh2_t[tt] = ohx
                xrow = gx.tile([128, TPC, XW], bf16, tag="xrow")
                nc.gpsimd.dma_gather(
                    xrow[:], agall[:], src2_t[:, ci * IC:(ci + 1) * IC],
                    CHUNK, CHUNK, XW)
                sl = slice(ci * TPC, (ci + 1) * TPC)
                z = sm.tile([128, TPC, 2], f32, tag="z2")
                nc.vector.tensor_tensor(
                    z[:],
                    xrow[:, :, 256:264].bitcast(f32)
                    .rearrange("p t (g s) -> p t g s", s=2)[:, :, :, 0],
                    sdall[:, sl, :].bitcast(f32)
                    .rearrange("p t (g s) -> p t g s", s=2)[:, :, :, 1],
                    op=Alu.add)
                nc.vector.scalar_tensor_tensor(
                    z[:], in0=z[:], scalar=NEG, in1=z[:],
                    op0=Alu.mult, op1=Alu.max)
                ex = sm.tile([128, TPC, 2], f32, tag="ex2")
                nc.scalar.activation(ex[:], z[:], Act.Exp)
                exw = sm.tile([128, TPC, 2], f32, tag="exw2")
                wb = wT_t[:, ci * TPC:(ci + 1) * TPC]
                nc.vector.tensor_tensor(
                    exw[:], ex[:],
                    wb.rearrange("p (t o) -> p t o", o=1).to_broadcast(
                        [128, TPC, 2]), op=Alu.mult)
                exw2 = sm.tile([128, TPC, 2, 2], bf16, tag="exw22")
                nc.vector.tensor_copy(
                    exw2[:], exw[:].rearrange("p t (g o) -> p t g o", o=1)
                    .to_broadcast([128, TPC, 2, 2]))
                xrg = xrow[:, :, 0:256].rearrange(
                    "p t (g k two) -> p t g k two", g=2, two=2)
                nc.vector.tensor_tensor(
                    xrg, xrg,
                    exw2[:].rearrange("p t g (o two) -> p t g o two", two=2)
                    .to_broadcast([128, TPC, 2, 64, 2]), op=Alu.mult)
                nc.vector.tensor_copy(xrow[:, :, 256:258], ex[:])
                nc.vector.tensor_tensor(xrow[:, :, 258:260], ex[:],
                                        xrow[:, :, 256:258], op=Alu.subtract)

                for tt in range(TPC):
                    t = ci * TPC + tt
                    if t >= TILES:
                        continue
                    b = t // TPB
                    k = t % TPB
                    if k == 0:
                        blk2[b] = ps2.tile([128, 260], f32, tag="blk",
                                           name="blk2ps")
                    ps2t = blk2[b]
                    nc.tensor.matmul(
                        ps2t[:, 0:260], oh2_t[tt][:], xrow[:, tt, 0:260],
                        start=(k == 0), stop=(k == TPB - 1))
                    if k == TPB - 1:
                        for li, (bias_t, outdr) in enumerate(
                                ((bmub_t, mu_out), (blvb_t, lv_out))):
                            den4 = sm.tile([128, 4], f32, tag="den4")
                            nc.vector.tensor_copy(den4[:], ps2t[:, 256:260])
                            den = sm.tile([128, 1], f32, tag="den2")
                            nc.vector.tensor_tensor(
                                den[:], den4[:, li:li + 1],
                                den4[:, 2 + li:3 + li], op=Alu.add)
                            nc.vector.tensor_scalar_add(den[:], den[:], EPS)
                            rec = sm.tile([128, 1], f32, tag="rec2")
                            nc.vector.reciprocal(rec[:], den[:])
                            ob = fin.tile([128, 128], f32, tag="ob")
                            nc.vector.scalar_tensor_tensor(
                                ob[:], in0=ps2t[:, li * 128:(li + 1) * 128],
                                scalar=rec[:, 0:1],
                                in1=bias_t[:], op0=Alu.mult, op1=Alu.add)
                            nc.sync.dma_start(
                                outdr[b * 128:(b + 1) * 128, :], ob[:])
                        del blk2[b]

    nc.compile()
    return nc


def _prep_inputs(x, edge_index, edge_weight, W1, att1, b1, Wmu, attmu, bmu,
                 Wlv, attlv, blv):
    import ml_dtypes
    bf = ml_dtypes.bfloat16

    src = np.asarray(edge_index[0], np.int64)
    dst = np.asarray(edge_index[1], np.int64)
    w = np.asarray(edge_weight, np.float32)
    x = np.asarray(x, np.float32)

    # fused weights
    att1 = np.asarray(att1, np.float32)          # [H, 2*C1]
    W1 = np.asarray(W1, np.float32)
    Wss1 = np.zeros((FIN, H), np.float32)
    Wsd1 = np.zeros((FIN, H), np.float32)
    for h in range(H):
        Wss1[:, h] = W1[:, h * C1:(h + 1) * C1] @ att1[h, C1:]
        Wsd1[:, h] = W1[:, h * C1:(h + 1) * C1] @ att1[h, :C1]
    w1e = np.concatenate([W1, Wss1, Wsd1], axis=1).astype(bf)   # [512, 264]

    attmu = np.asarray(attmu, np.float32).reshape(-1)
    attlv = np.asarray(attlv, np.float32).reshape(-1)
    Wmu = np.asarray(Wmu, np.float32)
    Wlv = np.asarray(Wlv, np.float32)
    wmue = np.concatenate(
        [Wmu, (Wmu @ attmu[LAT:])[:, None], (Wmu @ attmu[:LAT])[:, None]],
        axis=1).astype(bf)
    wlve = np.concatenate(
        [Wlv, (Wlv @ attlv[LAT:])[:, None], (Wlv @ attlv[:LAT])[:, None]],
        axis=1).astype(bf)

    xT_all = x.T.astype(bf)
    b1b = np.tile(np.asarray(b1, np.float32)[None, :], (128, 1))
    bmub = np.tile(np.asarray(bmu, np.float32)[None, :], (128, 1))
    blvb = np.tile(np.asarray(blv, np.float32)[None, :], (128, 1))
    iota = np.tile(np.arange(128, dtype=np.float32)[None, :],
                   (128, 1)).astype(bf)
    ident = np.eye(128, dtype=np.float32)

    # sort edges by dst, bucket per core, pad per 128-node block to TPB tiles
    order = np.argsort(dst, kind="stable")
    ssrc, sdst, sw = src[order], dst[order], w[order]
    core_of = sdst // NOWN
    in_maps = []
    for c in range(NC):
        m = core_of == c
        cs, cd, cw = ssrc[m], sdst[m] - c * NOWN, sw[m]
        blk = cd // 128
        e_src = np.zeros(EPAD, np.int64)
        e_dstloc = np.zeros(EPAD, np.int64)
        e_dstoff = np.full(EPAD, -1.0, np.float32)
        e_w = np.zeros(EPAD, np.float32)
        for b in range(BLOCKS):
            bm = blk == b
            nbe = int(bm.sum())
            if nbe > TPB * 128:
                raise RuntimeError(f"block overflow core {c} block {b}: {nbe}")
            o = b * TPB * 128
            e_src[o:o + nbe] = cs[bm]
            e_dstloc[o:o + nbe] = cd[bm]
            e_dstoff[o:o + nbe] = (cd[bm] - b * 128).astype(np.float32)
            e_w[o:o + nbe] = cw[bm]
        own = e_src // NOWN
        e_src2 = own * NLOC + (e_src - own * NOWN)
        # per-core node permutation: own dst nodes first (rows 0:2500)
        perm = np.concatenate([
            np.arange(c * NOWN, (c + 1) * NOWN),
            np.arange(0, c * NOWN),
            np.arange((c + 1) * NOWN, N)])
        inv = np.empty(N, np.int64)
        inv[perm] = np.arange(N)
        xTb_c = np.zeros((FIN, NPADA), bf)
        xTb_c[:, :N] = xT_all[:, perm]
        in_maps.append({
            "xTb": xTb_c, "w1e": w1e, "wmue": wmue, "wlve": wlve, "b1b": b1b,
            "bmub": bmub, "blvb": blvb, "iota": iota, "ident": ident,
            "identb": ident.astype(bf),
            "srcg": _wrap_idxs(inv[e_src]),
            "src2": _wrap_idxs(e_src2), "dstl": _wrap_idxs(e_dstloc),
            "dstoffT": _colmajor(e_dstoff),
            "wT": _colmajor(e_w),
        })
    return in_maps


def kernel(x, edge_index, edge_weight, W1, att1, b1, Wmu, attmu, bmu,
           Wlv, attlv, blv):
    from concourse.bass_utils import run_bass_kernel_spmd

    if "nc" not in _cache:
        _cache["nc"] = _build_module()
    nc = _cache["nc"]
    in_maps = _prep_inputs(x, edge_index, edge_weight, W1, att1, b1,
                           Wmu, attmu, bmu, Wlv, attlv, blv)
    r = run_bass_kernel_spmd(nc, in_maps, list(range(NC)))
    mu = np.zeros((N, LAT), np.float32)
    lv = np.zeros((N, LAT), np.float32)
    for c in range(NC):
        mu[c * NOWN:(c + 1) * NOWN] = r.results[c]["mu_out"][:NOWN]
        lv[c * NOWN:(c + 1) * NOWN] = r.results[c]["lv_out"][:NOWN]
    return (mu, lv)


# revision 12
# speedup vs baseline: 1.2317x; 1.0798x over previous
"""GAT encoder (4-head 256 concat + mu/logvar 128) on 8 trn2 cores — v2.

Strategy (dst-range node sharding):
 - Host sorts edges by dst, buckets per core (2500 dst nodes each), pads each
   128-node block to TPB tiles of 128 edges; 2048-edge gather chunks.
 - Phase A (per core, redundant): one fused bf16 matmul xTb @ [W1|Wss1|Wsd1]
   per 128-node tile -> XPT rows [xp bf16 0:256 | ss1 f32 | sd1 f32] (768B).
 - L1 edge phase: 768B dma_gather of XPT rows by src (payload+ss), 256B
   sub-row gather of XPT[256:384] by dst (sd); softmax coefs scaled into the
   payload in place; per 128-edge tile a one-hot bf16 matmul accumulates
   messages + ex hi/lo denominator columns into a per-block PSUM.
 - L1 finalize per block: normalize, +bias, ELU (bf16) -> h; PE-transpose and
   project with [Wmu|vmu|umu]/[Wlv|vlv|ulv]; rows [xpmu|xplv|logits f32] go to
   agin. AllGather is split into 4 chunks of 5 blocks, each issued as soon as
   its blocks are final so the collective overlaps the L1 tail.
 - L2/3 edge phase: 768B gather of agout rows by src, 256B sub-row gather of
   agin[256:384] by dst (prefetched into a compact sd table during the AG
   tail); same one-hot trick, mu and lv share one gather/matmul per tile.
Outputs (mu, logvar) assembled host-side from per-core slices.
"""

import numpy as np

# ---- problem constants (hardcoded per contract) ----
N = 20000
E = 320000
FIN = 512
HID = 256
LAT = 128
H = 4
C1 = 64
NEG = 0.2
EPS = 1e-16

NC = 8
NOWN = 2500          # dst nodes per core
BLOCKS = 20          # 128-node blocks per core
NLOC = BLOCKS * 128  # 2560
TPB = 18             # tiles (128 edges) per block (max real block = 2174 edges)
TILES = BLOCKS * TPB       # 360 real tiles
TPC = 8                   # tiles per gather chunk
CHUNK = TPC * 128
IC = CHUNK // 16           # idx table cols per chunk
NCHUNK = 45
TILES_PAD = NCHUNK * TPC   # 360
EPAD = TILES_PAD * 128     # 47104 edge slots per core
NPADA = 160 * 128          # 20480 padded global rows
XW = 384                   # XPT/agin/agout row width in bf16 slots (768B)
GAG = 4                    # AllGather split count (5 blocks each)
AG_EMIT = (13, 24, 35, 44)  # chunk after which AG_g is emitted (deps: 5,11,16,22)

_cache = {}


def _wrap_idxs(idx):
    n = idx.shape[0]
    t = np.zeros((128, n // 16), np.int16)
    w = idx.reshape(n // 16, 16).T.astype(np.int16)
    for g in range(8):
        t[g * 16:(g + 1) * 16, :] = w
    return t


def _colmajor(a):
    # per-edge array [EPAD] -> [128, TILES_PAD] tile-column layout
    return np.ascontiguousarray(a.reshape(TILES_PAD, 128).T)


def _build_module(upto="full"):
    import concourse.bacc as bacc
    import concourse.mybir as mybir
    import concourse.tile as tile

    f32 = mybir.dt.float32
    bf16 = mybir.dt.bfloat16
    i16 = mybir.dt.int16
    Alu = mybir.AluOpType
    Act = mybir.ActivationFunctionType

    nc = bacc.Bacc("TRN2", target_bir_lowering=False, num_devices=NC)

    # ---- inputs ----
    xTb = nc.dram_tensor("xTb", [FIN, NPADA], bf16, kind="ExternalInput")
    w1e = nc.dram_tensor("w1e", [FIN, 264], bf16, kind="ExternalInput")
    wmue = nc.dram_tensor("wmue", [HID, 130], bf16, kind="ExternalInput")
    wlve = nc.dram_tensor("wlve", [HID, 130], bf16, kind="ExternalInput")
    b1b = nc.dram_tensor("b1b", [128, 256], f32, kind="ExternalInput")
    bmub = nc.dram_tensor("bmub", [128, 128], f32, kind="ExternalInput")
    blvb = nc.dram_tensor("blvb", [128, 128], f32, kind="ExternalInput")
    iota = nc.dram_tensor("iota", [128, 128], bf16, kind="ExternalInput")
    ident = nc.dram_tensor("ident", [128, 128], f32, kind="ExternalInput")
    identb = nc.dram_tensor("identb", [128, 128], bf16, kind="ExternalInput")
    srcg = nc.dram_tensor("srcg", [128, EPAD // 16], i16, kind="ExternalInput")
    src2 = nc.dram_tensor("src2", [128, EPAD // 16], i16, kind="ExternalInput")
    dstl = nc.dram_tensor("dstl", [128, EPAD // 16], i16, kind="ExternalInput")
    dstoffT = nc.dram_tensor("dstoffT", [128, TILES_PAD], f32, kind="ExternalInput")
    wT = nc.dram_tensor("wT", [128, TILES_PAD], f32, kind="ExternalInput")

    mu_out = nc.dram_tensor("mu_out", [NLOC, LAT], f32, kind="ExternalOutput")
    lv_out = nc.dram_tensor("lv_out", [NLOC, LAT], f32, kind="ExternalOutput")

    with tile.TileContext(nc) as tc:
        with (
            tc.tile_pool(name="cst", bufs=1) as cst,
            tc.tile_pool(name="lw", bufs=3) as lw,
            tc.tile_pool(name="xa", bufs=3) as xa,
            tc.tile_pool(name="gx", bufs=3) as gx,
            tc.tile_pool(name="ge", bufs=3) as ge,
            tc.tile_pool(name="oh", bufs=20) as ohp,
            tc.tile_pool(name="sm", bufs=6) as sm,
            tc.tile_pool(name="fin", bufs=3) as fin,
            tc.tile_pool(name="ps2", bufs=3, space="PSUM") as ps2,
            tc.tile_pool(name="psa", bufs=2, space="PSUM") as psa,
            tc.tile_pool(name="ps1", bufs=1, space="PSUM") as ps1,
            tc.tile_pool(name="dr", bufs=1, space="DRAM") as dr,
        ):
            # internal DRAM tables (aginc is the contiguous collective
            # input; wide tables keep 768B rows for gathers)
            XPT = dr.tile([NPADA, XW], bf16, tag="XPT")
            SDT = dr.tile([NLOC, 128], bf16, tag="SDT")
            SD2T = dr.tile([NLOC, 128], bf16, tag="SD2T")
            agin = dr.tile([NLOC, XW], bf16, tag="agin")
            agincs = []
            for g in range(GAG):
                aginc_g = dr.tile([NLOC // GAG, 264], bf16,
                                  tag=f"aginc{g}", name=f"aginc{g}")
                agincs.append(aginc_g)
            agall = dr.tile([NC * NLOC, XW], bf16, tag="agall")
            agouts = []
            for g in range(GAG):
                agout_g = dr.tile([NC * (NLOC // GAG), 264], bf16,
                                  tag=f"agout{g}", name=f"agout{g}",
                                  addr_space="Shared")
                agouts.append(agout_g)

            # resident constants
            w1e_t = []
            for kk in range(4):
                t = cst.tile([128, 264], bf16, tag=f"w1e{kk}")
                nc.sync.dma_start(t[:], w1e[kk * 128:(kk + 1) * 128, :])
                w1e_t.append(t)
            wmue_t = []
            wlve_t = []
            for kk in range(2):
                t = cst.tile([128, 130], bf16, tag=f"wmue{kk}")
                nc.sync.dma_start(t[:], wmue[kk * 128:(kk + 1) * 128, :])
                wmue_t.append(t)
                t2 = cst.tile([128, 130], bf16, tag=f"wlve{kk}")
                nc.sync.dma_start(t2[:], wlve[kk * 128:(kk + 1) * 128, :])
                wlve_t.append(t2)
            b1b_t = cst.tile([128, 256], f32, tag="b1b")
            nc.sync.dma_start(b1b_t[:], b1b[:])
            bmub_t = cst.tile([128, 128], f32, tag="bmub")
            nc.sync.dma_start(bmub_t[:], bmub[:])
            blvb_t = cst.tile([128, 128], f32, tag="blvb")
            nc.sync.dma_start(blvb_t[:], blvb[:])
            iota_t = cst.tile([128, 128], bf16, tag="iota")
            nc.sync.dma_start(iota_t[:], iota[:])
            ident_t = cst.tile([128, 128], f32, tag="ident")
            nc.sync.dma_start(ident_t[:], ident[:])
            identb_t = cst.tile([128, 128], bf16, tag="identb")
            nc.sync.dma_start(identb_t[:], identb[:])
            srcg_t = cst.tile([128, EPAD // 16], i16, tag="srcg")
            nc.sync.dma_start(srcg_t[:], srcg[:])
            src2_t = cst.tile([128, EPAD // 16], i16, tag="src2")
            nc.sync.dma_start(src2_t[:], src2[:])
            dstl_t = cst.tile([128, EPAD // 16], i16, tag="dstl")
            nc.sync.dma_start(dstl_t[:], dstl[:])
            dstoffT_t = cst.tile([128, TILES_PAD], f32, tag="dstoffT")
            nc.sync.dma_start(dstoffT_t[:], dstoffT[:])
            wT_t = cst.tile([128, TILES_PAD], f32, tag="wT")
            nc.sync.dma_start(wT_t[:], wT[:])
            # compact per-edge dst logits for L2/3, filled during the AG tail
            sdall = cst.tile([128, TILES_PAD, 8], bf16, tag="sdall")
            # compact per-edge dst logits for L1, filled during phase A
            sdall1 = cst.tile([128, TILES_PAD, 8], bf16, tag="sdall1")

            # ---- phase A (replicated, own-first row permutation): one
            # fused matmul per 128-node tile -> XPT rows; own rows land in
            # groups 0-4 so L1 dst-logit gathers overlap later groups ----
            pre1_done = 0
            for g in range(NPADA // 512):
                lx = lw.tile([128, 4, 512], bf16, tag="lx")
                nc.sync.dma_start(
                    lx[:], xTb[:].rearrange("(kk p) (g n) -> p kk g n",
                                            p=128, n=512)[:, :, g, :])
                xps = xa.tile([128, 4, 272], bf16, tag="xps")
                for ti in range(4):
                    ps = psa.tile([128, 264], f32, tag="psA", name="psA")
                    for kk in range(4):
                        sl = slice(ti * 128, (ti + 1) * 128)
                        nc.tensor.matmul(ps[:], lx[:, kk, sl], w1e_t[kk][:],
                                         start=(kk == 0), stop=(kk == 3))
                    nc.scalar.copy(xps[:, ti, 0:256], ps[:, 0:256])
                    nc.vector.tensor_copy(
                        xps[:, ti, 256:272].bitcast(f32), ps[:, 256:264])
                nc.sync.dma_start(
                    XPT[:].rearrange("(g4 p) c -> p g4 c", p=128)
                    [:, 4 * g:4 * g + 4, 0:272], xps[:])
                if g < 5:
                    nc.sync.dma_start(
                        SDT[:].rearrange("(g4 p) c -> p g4 c", p=128)
                        [:, 4 * g:4 * g + 4, 0:16], xps[:, :, 256:272])
                if g >= 6 and upto != "A":
                    for _ in range(1):
                        if pre1_done >= 15:
                            break
                        ci1 = pre1_done
                        ext1 = ge.tile([128, TPC, 128], bf16, tag="ext")
                        nc.gpsimd.dma_gather(
                            ext1[:], SDT[:],
                            dstl_t[:, ci1 * IC:(ci1 + 1) * IC],
                            CHUNK, CHUNK, 128)
                        nc.vector.tensor_copy(
                            sdall1[:, ci1 * TPC:(ci1 + 1) * TPC, :],
                            ext1[:, :, 8:16])
                        pre1_done += 1

            # ---- L1 edge phase + finalize (+ split AllGather) ----
            blk_ps = {}
            ag_done = 0
            pre_done = 0

            GR = NLOC // GAG

            def emit_ag(g):
                nc.gpsimd.collective_compute(
                    "AllGather", mybir.AluOpType.bypass,
                    replica_groups=[list(range(NC))],
                    ins=[agincs[g][:]],
                    outs=[agouts[g][:]])

            for ci in range(NCHUNK if upto != "A" else 0):
                # one-hot builds first: they have no gather dependency, so
                # DVE chews them while this chunk's gathers are in flight
                ohx_t = {}
                for tt in range(TPC):
                    t = ci * TPC + tt
                    if t >= TILES:
                        continue
                    ohx = ohp.tile([128, 128], bf16, tag="ohx")
                    nc.vector.tensor_scalar(
                        ohx[:], iota_t[:], dstoffT_t[:, t:t + 1], None,
                        Alu.is_equal)
                    ohx_t[tt] = ohx
                xrow = gx.tile([128, TPC, XW], bf16, tag="xrow")
                nc.gpsimd.dma_gather(
                    xrow[:], XPT[:], srcg_t[:, ci * IC:(ci + 1) * IC],
                    CHUNK, CHUNK, XW)
                if pre1_done < NCHUNK and upto != "A":
                    ci1 = pre1_done
                    ext1 = ge.tile([128, TPC, 128], bf16, tag="ext")
                    nc.gpsimd.dma_gather(
                        ext1[:], SDT[:],
                        dstl_t[:, ci1 * IC:(ci1 + 1) * IC],
                        CHUNK, CHUNK, 128)
                    nc.vector.tensor_copy(
                        sdall1[:, ci1 * TPC:(ci1 + 1) * TPC, :],
                        ext1[:, :, 8:16])
                    pre1_done += 1
                # alpha: z = ss[src]+sd[dst], leaky, exp, *w
                z = sm.tile([128, TPC, 4], f32, tag="z")
                nc.vector.tensor_tensor(
                    z[:], xrow[:, :, 256:264].bitcast(f32),
                    sdall1[:, ci * TPC:(ci + 1) * TPC, :].bitcast(f32),
                    op=Alu.add)
                nc.vector.scalar_tensor_tensor(
                    z[:], in0=z[:], scalar=NEG, in1=z[:],
                    op0=Alu.mult, op1=Alu.max)
                ex = sm.tile([128, TPC, 4], f32, tag="ex")
                nc.scalar.activation(ex[:], z[:], Act.Exp)
                exw = sm.tile([128, TPC, 4], f32, tag="exw")
                wb = wT_t[:, ci * TPC:(ci + 1) * TPC]
                nc.vector.tensor_tensor(
                    exw[:], ex[:],
                    wb.rearrange("p (t o) -> p t o", o=1).to_broadcast(
                        [128, TPC, 4]), op=Alu.mult)
                # pair-packed copy of exw so the big scale op runs in 2x mode
                exw2 = sm.tile([128, TPC, 4, 2], bf16, tag="exw2")
                nc.vector.tensor_copy(
                    exw2[:], exw[:].rearrange("p t (h o) -> p t h o", o=1)
                    .to_broadcast([128, TPC, 4, 2]))
                xrh = xrow[:, :, 0:256].rearrange(
                    "p t (h k two) -> p t h k two", h=4, two=2)
                nc.vector.tensor_tensor(
                    xrh, xrh,
                    exw2[:].rearrange("p t h (o two) -> p t h o two", two=2)
                    .to_broadcast([128, TPC, 4, 32, 2]), op=Alu.mult)
                # unweighted ex -> hi/lo bf16 denominator cols 256:264
                nc.vector.tensor_copy(xrow[:, :, 256:260], ex[:])
                nc.vector.tensor_tensor(xrow[:, :, 260:264], ex[:],
                                        xrow[:, :, 256:260], op=Alu.subtract)

                for tt in range(TPC):
                    t = ci * TPC + tt
                    if t >= TILES:
                        continue
                    b = t // TPB
                    k = t % TPB
                    if k == 0:
                        blk_ps[b] = ps2.tile([128, 264], f32, tag="blk",
                                             name="blkps")
                    ps = blk_ps[b]
                    nc.tensor.matmul(
                        ps[:, 0:264], ohx_t[tt][:], xrow[:, tt, 0:264],
                        start=(k == 0), stop=(k == TPB - 1))
                    if k == TPB - 1:
                        # finalize block b -> h, then next-layer rows
                        den8 = sm.tile([128, 8], f32, tag="den8")
                        nc.vector.tensor_copy(den8[:], ps[:, 256:264])
                        den = sm.tile([128, 4], f32, tag="den")
                        nc.vector.tensor_tensor(den[:], den8[:, 0:4],
                                                den8[:, 4:8], op=Alu.add)
                        nc.vector.tensor_scalar_add(den[:], den[:], EPS)
                        rec = sm.tile([128, 4], f32, tag="rec")
                        nc.vector.reciprocal(rec[:], den[:])
                        hb = fin.tile([128, 256], f32, tag="hb")
                        for h in range(H):
                            nc.vector.scalar_tensor_tensor(
                                hb[:, h * 64:(h + 1) * 64],
                                in0=ps[:, h * 64:(h + 1) * 64],
                                scalar=rec[:, h:h + 1],
                                in1=b1b_t[:, h * 64:(h + 1) * 64],
                                op0=Alu.mult, op1=Alu.add)
                        # ELU: h = max(z,0) + exp(min(z,0)) - 1
                        zm = fin.tile([128, 256], f32, tag="zm")
                        nc.vector.tensor_scalar_min(zm[:], hb[:], 0.0)
                        ez = fin.tile([128, 256], f32, tag="ez")
                        nc.scalar.activation(ez[:], zm[:], Act.Exp)
                        nc.vector.scalar_tensor_tensor(
                            hb[:], in0=hb[:], scalar=0.0, in1=ez[:],
                            op0=Alu.max, op1=Alu.add)
                        nc.vector.tensor_scalar_add(hb[:], hb[:], -1.0)
                        # transpose h (2 x 128x128) and project
                        hTs = []
                        for half in range(2):
                            pst = ps1.tile([128, 128], f32, tag="pst")
                            nc.tensor.transpose(
                                pst[:], hb[:, half * 128:(half + 1) * 128],
                                ident_t[:])
                            hT = fin.tile([128, 128], bf16, tag=f"hT{half}")
                            nc.vector.tensor_copy(hT[:], pst[:])
                            hTs.append(hT)
                        psmu_t = ps1.tile([128, 130], f32, tag="psmu")
                        pslv_t = ps1.tile([128, 130], f32, tag="pslv")
                        psmu = psmu_t[:]
                        pslv = pslv_t[:]
                        for kk in range(2):
                            nc.tensor.matmul(psmu, hTs[kk][:], wmue_t[kk][:],
                                             start=(kk == 0), stop=(kk == 1))
                            nc.tensor.matmul(pslv, hTs[kk][:], wlve_t[kk][:],
                                             start=(kk == 0), stop=(kk == 1))
                        # agin row: [xpmu | xplv | ssmu sdmu sslv sdlv (f32)]
                        xr2 = fin.tile([128, 264], bf16, tag="xr2")
                        nc.scalar.copy(xr2[:, 0:128], psmu[:, 0:128])
                        nc.scalar.copy(xr2[:, 128:256], pslv[:, 0:128])
                        ssv = xr2[:, 256:264].bitcast(f32)
                        nc.vector.tensor_copy(ssv[:, 0:2], psmu[:, 128:130])
                        nc.vector.tensor_copy(ssv[:, 2:4], pslv[:, 128:130])
                        nc.sync.dma_start(
                            agin[b * 128:(b + 1) * 128, 0:264], xr2[:])
                        gb = b // (BLOCKS // GAG)
                        rb = b % (BLOCKS // GAG)
                        nc.sync.dma_start(
                            agincs[gb][rb * 128:(rb + 1) * 128, :], xr2[:])
                        nc.sync.dma_start(
                            SD2T[b * 128:(b + 1) * 128, 0:8], xr2[:, 256:264])
                        del blk_ps[b]

                if upto in ("AG", "full"):
                    while ag_done < GAG and ci >= AG_EMIT[ag_done]:
                        emit_ag(ag_done)
                        ag_done += 1
                # L2/3 dst-logit prefetches whose blocks are final (bounded
                # row range keeps the Pool-queue hold near zero)
                if upto == "full":
                    fin_blocks = ((ci - 2) * TPC + TPC - 1) // TPB if ci > 1 else -1
                    while (pre_done < NCHUNK
                           and (pre_done * TPC + TPC - 1) // TPB < fin_blocks):
                        ci2 = pre_done
                        hib = min((ci2 * TPC + TPC - 1) // TPB, BLOCKS - 1)
                        ext2 = ge.tile([128, TPC, 128], bf16, tag="ext")
                        nc.gpsimd.dma_gather(
                            ext2[:], SD2T[0:(hib + 1) * 128, :],
                            dstl_t[:, ci2 * IC:(ci2 + 1) * IC],
                            CHUNK, CHUNK, 128)
                        nc.vector.tensor_copy(
                            sdall[:, ci2 * TPC:(ci2 + 1) * TPC, :],
                            ext2[:, :, 0:8])
                        pre_done += 1

            # ---- repack compact AG outputs into the 768B-stride table ----
            # (wait_until_ts keeps the scheduler from hoisting these into the
            # middle of L1 where their collective-wait would hold the queue)
            if upto in ("AG", "full"):
                for g in range(GAG):
                    rp = nc.scalar.dma_start(
                        agall[:].rearrange("(c r) w -> c r w", r=NLOC)
                        [:, g * GR:(g + 1) * GR, 0:264],
                        agouts[g][:].rearrange("(c r) w -> c r w", r=GR))
                    rp.ins.bass_wait_until_ts = REPACK_TS[g]

            # ---- remaining L2/3 dst-logit prefetches ----
            for ci in range(pre_done, NCHUNK if upto == "full" else 0):
                ext2 = ge.tile([128, TPC, 128], bf16, tag="ext")
                nc.gpsimd.dma_gather(
                    ext2[:], SD2T[:], dstl_t[:, ci * IC:(ci + 1) * IC],
                    CHUNK, CHUNK, 128)
                nc.vector.tensor_copy(
                    sdall[:, ci * TPC:(ci + 1) * TPC, :], ext2[:, :, 0:8])

            # ---- L2/3 edge phase (mu and lv share gathers) ----
            OHPRE = 10
            oh_pre = {}
            for ci in range(OHPRE if upto == "full" else 0):
                for tt in range(TPC):
                    t = ci * TPC + tt
                    if t >= TILES:
                        continue
                    ohx = ohp.tile([128, 128], bf16, tag="ohx2",
                                   bufs=OHPRE * TPC + 2)
                    eng = nc.vector if tt % 2 == 0 else nc.gpsimd
                    eng.tensor_scalar(
                        ohx[:], iota_t[:], dstoffT_t[:, t:t + 1], None,
                        Alu.is_equal)
                    oh_pre[t] = ohx
            blk2 = {}
            for ci in range(NCHUNK if upto == "full" else 0):
                oh2_t = {}
                for tt in range(TPC):
                    t = ci * TPC + tt
                    if t >= TILES:
                        continue
                    if t in oh_pre:
                        oh2_t[tt] = oh_pre.pop(t)
                        continue
                    ohx = ohp.tile([128, 128], bf16, tag="ohx")
                    eng = nc.vector if tt % 2 == 0 else nc.gpsimd
                    eng.tensor_scalar(
                        ohx[:], iota_t[:], dstoffT_t[:, t:t + 1], None,
                        Alu.is_equal)
                    oh2_t[tt] = ohx
                xrow = gx.tile([128, TPC, XW], bf16, tag="xrow")
                nc.gpsimd.dma_gather(
                    xrow[:], agall[:], src2_t[:, ci * IC:(ci + 1) * IC],
                    CHUNK, CHUNK, XW)
                sl = slice(ci * TPC, (ci + 1) * TPC)
                z = sm.tile([128, TPC, 2], f32, tag="z2")
                nc.vector.tensor_tensor(
                    z[:],
                    xrow[:, :, 256:264].bitcast(f32)
                    .rearrange("p t (g s) -> p t g s", s=2)[:, :, :, 0],
                    sdall[:, sl, :].bitcast(f32)
                    .rearrange("p t (g s) -> p t g s", s=2)[:, :, :, 1],
                    op=Alu.add)
                nc.vector.scalar_tensor_tensor(
                    z[:], in0=z[:], scalar=NEG, in1=z[:],
                    op0=Alu.mult, op1=Alu.max)
                ex = sm.tile([128, TPC, 2], f32, tag="ex2")
                nc.scalar.activation(ex[:], z[:], Act.Exp)
                exw = sm.tile([128, TPC, 2], f32, tag="exw2")
                wb = wT_t[:, ci * TPC:(ci + 1) * TPC]
                nc.vector.tensor_tensor(
                    exw[:], ex[:],
                    wb.rearrange("p (t o) -> p t o", o=1).to_broadcast(
                        [128, TPC, 2]), op=Alu.mult)
                exw2 = sm.tile([128, TPC, 2, 2], bf16, tag="exw22")
                nc.vector.tensor_copy(
                    exw2[:], exw[:].rearrange("p t (g o) -> p t g o", o=1)
                    .to_broadcast([128, TPC, 2, 2]))
                xrg = xrow[:, :, 0:256].rearrange(
                    "p t (g k two) -> p t g k two", g=2, two=2)
                nc.vector.tensor_tensor(
                    xrg, xrg,
                    exw2[:].rearrange("p t g (o two) -> p t g o two", two=2)
                    .to_broadcast([128, TPC, 2, 64, 2]), op=Alu.mult)
                nc.vector.tensor_copy(xrow[:, :, 256:258], ex[:])
                nc.vector.tensor_tensor(xrow[:, :, 258:260], ex[:],
                                        xrow[:, :, 256:258], op=Alu.subtract)

                for tt in range(TPC):
                    t = ci * TPC + tt
                    if t >= TILES:
                        continue
                    b = t // TPB
                    k = t % TPB
                    if k == 0:
                        blk2[b] = ps2.tile([128, 260], f32, tag="blk",
                                           name="blk2ps")
                    ps2t = blk2[b]
                    nc.tensor.matmul(
                        ps2t[:, 0:260], oh2_t[tt][:], xrow[:, tt, 0:260],
                        start=(k == 0), stop=(k == TPB - 1))
                    if k == TPB - 1:
                        for li, (bias_t, outdr) in enumerate(
                                ((bmub_t, mu_out), (blvb_t, lv_out))):
                            den4 = sm.tile([128, 4], f32, tag="den4")
                            nc.vector.tensor_copy(den4[:], ps2t[:, 256:260])
                            den = sm.tile([128, 1], f32, tag="den2")
                            nc.vector.tensor_tensor(
                                den[:], den4[:, li:li + 1],
                                den4[:, 2 + li:3 + li], op=Alu.add)
                            nc.vector.tensor_scalar_add(den[:], den[:], EPS)
                            rec = sm.tile([128, 1], f32, tag="rec2")
                            nc.vector.reciprocal(rec[:], den[:])
                            ob = fin.tile([128, 128], f32, tag="ob")
                            nc.vector.scalar_tensor_tensor(
                                ob[:], in0=ps2t[:, li * 128:(li + 1) * 128],
                                scalar=rec[:, 0:1],
                                in1=bias_t[:], op0=Alu.mult, op1=Alu.add)
                            nc.sync.dma_start(
                                outdr[b * 128:(b + 1) * 128, :], ob[:])
                        del blk2[b]

    nc.compile()
    return nc


def _prep_inputs(x, edge_index, edge_weight, W1, att1, b1, Wmu, attmu, bmu,
                 Wlv, attlv, blv):
    import ml_dtypes
    bf = ml_dtypes.bfloat16

    src = np.asarray(edge_index[0], np.int64)
    dst = np.asarray(edge_index[1], np.int64)
    w = np.asarray(edge_weight, np.float32)
    x = np.asarray(x, np.float32)

    # fused weights
    att1 = np.asarray(att1, np.float32)          # [H, 2*C1]
    W1 = np.asarray(W1, np.float32)
    Wss1 = np.zeros((FIN, H), np.float32)
    Wsd1 = np.zeros((FIN, H), np.float32)
    for h in range(H):
        Wss1[:, h] = W1[:, h * C1:(h + 1) * C1] @ att1[h, C1:]
        Wsd1[:, h] = W1[:, h * C1:(h + 1) * C1] @ att1[h, :C1]
    w1e = np.concatenate([W1, Wss1, Wsd1], axis=1).astype(bf)   # [512, 264]

    attmu = np.asarray(attmu, np.float32).reshape(-1)
    attlv = np.asarray(attlv, np.float32).reshape(-1)
    Wmu = np.asarray(Wmu, np.float32)
    Wlv = np.asarray(Wlv, np.float32)
    wmue = np.concatenate(
        [Wmu, (Wmu @ attmu[LAT:])[:, None], (Wmu @ attmu[:LAT])[:, None]],
        axis=1).astype(bf)
    wlve = np.concatenate(
        [Wlv, (Wlv @ attlv[LAT:])[:, None], (Wlv @ attlv[:LAT])[:, None]],
        axis=1).astype(bf)

    xT_all = x.T.astype(bf)
    b1b = np.tile(np.asarray(b1, np.float32)[None, :], (128, 1))
    bmub = np.tile(np.asarray(bmu, np.float32)[None, :], (128, 1))
    blvb = np.tile(np.asarray(blv, np.float32)[None, :], (128, 1))
    iota = np.tile(np.arange(128, dtype=np.float32)[None, :],
                   (128, 1)).astype(bf)
    ident = np.eye(128, dtype=np.float32)

    # sort edges by dst, bucket per core, pad per 128-node block to TPB tiles
    order = np.argsort(dst, kind="stable")
    ssrc, sdst, sw = src[order], dst[order], w[order]
    core_of = sdst // NOWN
    in_maps = []
    for c in range(NC):
        m = core_of == c
        cs, cd, cw = ssrc[m], sdst[m] - c * NOWN, sw[m]
        blk = cd // 128
        e_src = np.zeros(EPAD, np.int64)
        e_dstloc = np.zeros(EPAD, np.int64)
        e_dstoff = np.full(EPAD, -1.0, np.float32)
        e_w = np.zeros(EPAD, np.float32)
        for b in range(BLOCKS):
            bm = blk == b
            nbe = int(bm.sum())
            if nbe > TPB * 128:
                raise RuntimeError(f"block overflow core {c} block {b}: {nbe}")
            o = b * TPB * 128
            e_src[o:o + nbe] = cs[bm]
            e_dstloc[o:o + nbe] = cd[bm]
            e_dstoff[o:o + nbe] = (cd[bm] - b * 128).astype(np.float32)
            e_w[o:o + nbe] = cw[bm]
        own = e_src // NOWN
        e_src2 = own * NLOC + (e_src - own * NOWN)
        # per-core node permutation: own dst nodes first (rows 0:2500)
        perm = np.concatenate([
            np.arange(c * NOWN, (c + 1) * NOWN),
            np.arange(0, c * NOWN),
            np.arange((c + 1) * NOWN, N)])
        inv = np.empty(N, np.int64)
        inv[perm] = np.arange(N)
        xTb_c = np.zeros((FIN, NPADA), bf)
        xTb_c[:, :N] = xT_all[:, perm]
        in_maps.append({
            "xTb": xTb_c, "w1e": w1e, "wmue": wmue, "wlve": wlve, "b1b": b1b,
            "bmub": bmub, "blvb": blvb, "iota": iota, "ident": ident,
            "identb": ident.astype(bf),
            "srcg": _wrap_idxs(inv[e_src]),
            "src2": _wrap_idxs(e_src2), "dstl": _wrap_idxs(e_dstloc),
            "dstoffT": _colmajor(e_dstoff),
            "wT": _colmajor(e_w),
        })
    return in_maps


def kernel(x, edge_index, edge_weight, W1, att1, b1, Wmu, attmu, bmu,
           Wlv, attlv, blv):
    from concourse.bass_utils import run_bass_kernel_spmd

    if "nc" not in _cache:
        _cache["nc"] = _build_module()
    nc = _cache["nc"]
    in_maps = _prep_inputs(x, edge_index, edge_weight, W1, att1, b1,
                           Wmu, attmu, bmu, Wlv, attlv, blv)
    r = run_bass_kernel_spmd(nc, in_maps, list(range(NC)))
    mu = np.zeros((N, LAT), np.float32)
    lv = np.zeros((N, LAT), np.float32)
    for c in range(NC):
        mu[c * NOWN:(c + 1) * NOWN] = r.results[c]["mu_out"][:NOWN]
        lv[c * NOWN:(c + 1) * NOWN] = r.results[c]["lv_out"][:NOWN]
    return (mu, lv)
